# revision 1
# baseline (speedup 1.0000x reference)
"""Multi-head causal self-attention (torch nn.MultiheadAttention semantics)
on 8 Trainium2 NeuronCores.

Problem: x [2, 2048, 1024], 16 heads, head dim 64, fp32, causal, p_drop=0.

Sharding: 2 batch groups x 4-way head tensor-parallel.
  core c: batch b = c // 4, heads [lane*4, lane*4+4) with lane = c % 4.
Each core computes q/k/v projections for its 4 heads, flash-style causal
attention (S^T score layout, no-max softmax — scores are O(1) here), and its
partial out-projection. The host sums the 4 partials per batch and adds b_out
(this is the all-reduce of the tensor-parallel decomposition, done on host
since the harness contract is full-input -> full-output).

All matmuls run in f32r (reduced-precision fp32 mode of the PE): same
throughput as bf16 (1 cycle/row at moving free >= 256) with ~1.5e-4 matmul
relative error; end-to-end absmax rel err vs the fp32 reference is ~1e-4.

Per-core program details:
  qkT [2*DQ, S] = (wqkT.T @ xT) + bqk  (q and k kept transposed: [dh, seq])
  v' per sk-block: [128, 4*(64+1)] — per-head v with an appended ones column,
     so the PV matmul's row 64 accumulates the softmax denominator for free.
  scores^T block [sk 128, sq 512] = kT.T @ qT; P = exp(s/8) (f32r);
     diagonal blocks multiplied by a precomputed 0/1 causal mask;
  out^T psum [65, 512] accumulates v'.T @ P over sk blocks; row 64 = denom;
     normalized via reciprocal + gpsimd partition-broadcast + DVE mul.
  out [S, DM] partial = OT.T @ woT per 128-row block.
"""

import os
from contextlib import ExitStack
from dataclasses import dataclass

import numpy as np

import concourse.bass as bass
import concourse.tile as tile
from concourse import bacc, mybir
from concourse.bass_utils import run_bass_kernel_spmd

F32 = mybir.dt.float32
F32R = mybir.dt.float32r
AF = mybir.ActivationFunctionType

B = 2
S = 2048
DM = 1024
N_HEADS = 16
DH = 64
N_CORES = 8
CPG = 4  # cores per group (tensor-parallel width over heads)
HPC = N_HEADS // CPG  # heads per core
DQ = HPC * DH
SPAN = 512
SB = 128
NDM = DM // 128
NSPAN = S // SPAN
NSB = S // SB
SBS = SPAN // SB
NQK = 2 * DQ // 128
NHD = DQ // 128
VW = DH + 1
OW = min(512, DM)
NOUT = DM // OW


def _declare_io(nc):
    t = {}
    t["xT"] = nc.dram_tensor("xT", [DM, S], F32R, kind="ExternalInput").ap()
    t["wqkT"] = nc.dram_tensor("wqkT", [DM, 2 * DQ], F32R, kind="ExternalInput").ap()
    t["wvT"] = nc.dram_tensor("wvT", [DM, DQ], F32R, kind="ExternalInput").ap()
    t["woT"] = nc.dram_tensor("woT", [DQ, DM], F32R, kind="ExternalInput").ap()
    t["bqk"] = nc.dram_tensor("bqk", [2 * DQ, 1], F32, kind="ExternalInput").ap()
    t["bv"] = nc.dram_tensor("bv", [128, DQ], F32, kind="ExternalInput").ap()
    t["out"] = nc.dram_tensor("out", [S, DM], F32, kind="ExternalOutput").ap()
    return t


def _build(ctx: ExitStack, tc: tile.TileContext, io: dict):
    nc = tc.nc

    const = ctx.enter_context(tc.tile_pool(name="const", bufs=1))
    work = ctx.enter_context(tc.tile_pool(name="work", bufs=1))
    psum = ctx.enter_context(tc.tile_pool(name="psum", bufs=1, space="PSUM"))

    # ---- constants / inputs ----
    xT = [const.tile([128, S], F32R, name=f"xT{c}") for c in range(NDM)]
    for c in range(NDM):
        nc.sync.dma_start(xT[c][:], io["xT"][c * 128 : (c + 1) * 128, :])

    wqk = [const.tile([128, 2 * DQ], F32R, name=f"wqk{c}") for c in range(NDM)]
    for c in range(NDM):
        nc.sync.dma_start(wqk[c][:], io["wqkT"][c * 128 : (c + 1) * 128, :])

    wv = [const.tile([128, DQ], F32R, name=f"wv{c}") for c in range(NDM)]
    for c in range(NDM):
        nc.sync.dma_start(wv[c][:], io["wvT"][c * 128 : (c + 1) * 128, :])

    wo = [const.tile([128, DM], F32R, name=f"wo{c}") for c in range(NHD)]
    for c in range(NHD):
        nc.sync.dma_start(wo[c][:], io["woT"][c * 128 : (c + 1) * 128, :])

    bqk = [const.tile([128, 1], F32, name=f"bqk{c}") for c in range(NQK)]
    for c in range(NQK):
        nc.sync.dma_start(bqk[c][:], io["bqk"][c * 128 : (c + 1) * 128, :])

    bv = const.tile([128, DQ], F32, name="bv")
    nc.sync.dma_start(bv[:], io["bv"][:])

    # triangular causal mask for the diagonal 128x128 sub-block:
    # tri[r, c] = (c - r >= 0)
    tri = const.tile([128, 128], F32R, name="tri")
    nc.gpsimd.memset(tri[:].bitcast(F32), 1.0)
    nc.gpsimd.affine_select(
        out=tri[:].bitcast(F32),
        in_=tri[:].bitcast(F32),
        compare_op=mybir.AluOpType.is_ge,
        fill=0.0,
        base=0,
        pattern=[[1, 128]],
        channel_multiplier=-1,
    )

    # ---- phase 1: q/k projection (transposed layout) ----
    qkT = [const.tile([128, S], F32R, name=f"qkT{b}") for b in range(NQK)]
    for ob in range(NQK):
        for sp in range(NSPAN):
            pqk = psum.tile([128, SPAN], F32, name=f"pqk_{ob}_{sp}", tag="po", bufs=4)
            for c in range(NDM):
                nc.tensor.matmul(
                    pqk[:],
                    wqk[c][:, ob * 128 : (ob + 1) * 128],
                    xT[c][:, sp * SPAN : (sp + 1) * SPAN],
                    start=(c == 0),
                    stop=(c == NDM - 1),
                )
            nc.vector.tensor_scalar_add(
                qkT[ob][:, sp * SPAN : (sp + 1) * SPAN], pqk[:], bqk[ob][:]
            )

    # ---- phase 2: v projection into v' (per-head + ones column) ----
    vp = [const.tile([128, HPC * VW], F32R, name=f"vp{sb}") for sb in range(NSB)]
    for sb in range(NSB):
        pv = psum.tile([128, DQ], F32, name=f"pv_{sb}", tag="po", bufs=4)
        for c in range(NDM):
            nc.tensor.matmul(
                pv[:],
                xT[c][:, sb * 128 : (sb + 1) * 128],
                wv[c][:],
                start=(c == 0),
                stop=(c == NDM - 1),
            )
        vdst = vp[sb][:, 0 : HPC * VW].rearrange("p (h w) -> p h w", w=VW)[:, :, 0:DH]
        nc.vector.tensor_add(
            vdst,
            pv[:].rearrange("p (h d) -> p h d", d=DH),
            bv[:].rearrange("p (h d) -> p h d", d=DH),
        )
        ones_cols = vp[sb][:, DH : HPC * VW : VW]
        nc.vector.memset(ones_cols.bitcast(F32), 1.0)

    # ---- phase 3+4: attention (flash, S^T layout) + interleaved out-proj ----
    # Per sk-block group: all HPC heads' score matmuls (uniform K=64 shape),
    # then all HPC heads' PV matmuls (uniform K=128 shape, distinct PSUM
    # banks), PVs lagging one group so the exp chain stays off PE's critical
    # path. Shape-uniform runs keep the PE array from draining between
    # matmuls (alternating K=64/K=128 measured 672 ns/mm vs 232 uniform).
    OT = [const.tile([128, S], F32R, name=f"OT{c}") for c in range(NHD)]
    for sp in range(NSPAN):
        den = work.tile([32 * (HPC - 1) + 1, SPAN], F32, name=f"den_{sp}", tag="den", bufs=1)
        nsb = (sp + 1) * SBS  # causal: sk blocks up to the span end
        pos = {}
        pts = {}
        oraw = {}

        def emit_scores(sb):
            for h in range(HPC):
                qt = qkT[h // 2]
                kt = qkT[NQK // 2 + h // 2]
                qrow = (h % 2) * 64
                ps = psum.tile(
                    [128, SPAN], F32, name=f"ps_{h}_{sp}_{sb}", tag="ps", bufs=4
                )
                nc.tensor.matmul(
                    ps[:],
                    kt[qrow : qrow + 64, sb * 128 : (sb + 1) * 128],
                    qt[qrow : qrow + 64, sp * SPAN : (sp + 1) * SPAN],
                    start=True,
                    stop=True,
                )
                pt = work.tile(
                    [128, SPAN], F32R, name=f"pt_{h}_{sp}_{sb}", tag="pt", bufs=8
                )
                pts[(h, sb)] = pt
                d = sb - sp * SBS
                if d < 0:
                    nc.scalar.activation(pt[:], ps[:], AF.Exp, scale=0.125)
                else:
                    # diagonal block: cols < 128*d fully masked, then one
                    # triangular 128x128 sub-block
                    if d > 0:
                        nc.vector.memset(pt[:, 0 : 128 * d].bitcast(F32), 0.0)
                    nc.scalar.activation(
                        pt[:, 128 * d : SPAN], ps[:, 128 * d : SPAN],
                        AF.Exp, scale=0.125,
                    )
                    nc.vector.tensor_mul(
                        pt[:, 128 * d : 128 * (d + 1)],
                        pt[:, 128 * d : 128 * (d + 1)],
                        tri[:],
                    )

        def emit_pvs(sb):
            for h in range(HPC):
                if sb == 0:
                    pos[h] = psum.tile(
                        [VW, SPAN], F32, name=f"po_{h}_{sp}", tag="po", bufs=4
                    )
                nc.tensor.matmul(
                    pos[h][:],
                    vp[sb][:, h * VW : (h + 1) * VW],
                    pts.pop((h, sb))[:],
                    start=(sb == 0),
                    stop=(sb == nsb - 1),
                )
                if sb == nsb - 1:
                    # copy (out^T | denom) to SBUF to free the PSUM bank early
                    orw = work.tile(
                        [VW, SPAN], F32, name=f"oraw_{h}_{sp}", tag="oraw", bufs=4
                    )
                    oraw[h] = orw
                    nc.vector.tensor_copy(orw[:], pos[h][:])
                    nc.vector.tensor_copy(den[32 * h : 32 * h + 1, :], orw[VW - 1 : VW, :])

        for i in range(nsb + 1):
            if i < nsb:
                emit_scores(i)
            if i >= 1:
                emit_pvs(i - 1)

        denr = work.tile([32 * (HPC - 1) + 1, SPAN], F32, name=f"denr_{sp}", tag="denr", bufs=1)
        # only rows 0/32/64/96 are meaningful; reciprocal of the garbage
        # rows in between is never read
        nc.vector.reciprocal(denr[:], den[:])
        for h in range(HPC):
            ot_tile = OT[(h * DH) // 128]
            orow = (h * DH) % 128
            rtmp = work.tile([1, SPAN], F32, name=f"rtmp_{h}_{sp}", tag="rtmp", bufs=2)
            # partition_broadcast needs a partition-0 source
            nc.vector.tensor_copy(rtmp[:], denr[32 * h : 32 * h + 1, :])
            recb = work.tile([DH, SPAN], F32, name=f"recb_{h}_{sp}", tag="recb", bufs=2)
            nc.gpsimd.partition_broadcast(recb[:], rtmp[0:1, :])
            nc.vector.tensor_mul(
                ot_tile[orow : orow + DH, sp * SPAN : (sp + 1) * SPAN],
                oraw[h][0:DH, :],
                recb[:],
            )
    # out projection for this span's sq blocks
        for qb in range(sp * SBS, (sp + 1) * SBS):
            ob = work.tile([128, DM], F32, name=f"ob_{qb}", tag="ob", bufs=2)
            for nh in range(NOUT):
                pot = psum.tile([128, OW], F32, name=f"pot_{qb}_{nh}", tag="po", bufs=4)
                for c in range(NHD):
                    nc.tensor.matmul(
                        pot[:],
                        OT[c][:, qb * 128 : (qb + 1) * 128],
                        wo[c][:, nh * OW : (nh + 1) * OW],
                        start=(c == 0),
                        stop=(c == NHD - 1),
                    )
                if (qb + nh) % 2 == 0:
                    nc.scalar.copy(ob[:, nh * OW : (nh + 1) * OW], pot[:])
                else:
                    nc.vector.tensor_copy(ob[:, nh * OW : (nh + 1) * OW], pot[:])
            nc.sync.dma_start(io["out"][qb * 128 : (qb + 1) * 128, :], ob[:])


_NC_CACHE = {}


def _get_compiled():
    if "nc" not in _NC_CACHE:
        nc = bacc.Bacc(
            "TRN2", target_bir_lowering=False, debug=False, num_devices=N_CORES
        )
        io = _declare_io(nc)
        with tile.TileContext(nc) as tc, ExitStack() as ctx:
            _build(ctx, tc, io)
        nc.compile()
        _NC_CACHE["nc"] = nc
    return _NC_CACHE["nc"]


def _prep_core_inputs(x, W_qkv, b_qkv, W_out, b_out, core_id):
    g = core_id // CPG
    lane = core_id % CPG
    h0 = lane * HPC
    r = slice(h0 * DH, (h0 + HPC) * DH)
    Wq = W_qkv[0 * DM : 1 * DM, :][r, :]
    Wk = W_qkv[1 * DM : 2 * DM, :][r, :]
    Wv = W_qkv[2 * DM : 3 * DM, :][r, :]
    bq = b_qkv[0 * DM + h0 * DH : 0 * DM + (h0 + HPC) * DH]
    bk = b_qkv[1 * DM + h0 * DH : 1 * DM + (h0 + HPC) * DH]
    bv_ = b_qkv[2 * DM + h0 * DH : 2 * DM + (h0 + HPC) * DH]
    return {
        "xT": np.ascontiguousarray(x[g].T.astype(np.float32)),
        "wqkT": np.ascontiguousarray(
            np.concatenate([Wq.T, Wk.T], axis=1).astype(np.float32)
        ),
        "wvT": np.ascontiguousarray(Wv.T.astype(np.float32)),
        "woT": np.ascontiguousarray(W_out[:, r].T.astype(np.float32)),
        "bqk": np.concatenate([bq, bk]).reshape(2 * DQ, 1).astype(np.float32),
        "bv": np.ascontiguousarray(
            np.broadcast_to(bv_.reshape(1, DQ), (128, DQ)).astype(np.float32)
        ),
    }


def kernel(x, W_qkv, b_qkv, W_out, b_out, _trace=False):
    x = np.asarray(x)
    W_qkv = np.asarray(W_qkv)
    b_qkv = np.asarray(b_qkv)
    W_out = np.asarray(W_out)
    b_out = np.asarray(b_out)

    nc = _get_compiled()
    in_maps = [
        _prep_core_inputs(x, W_qkv, b_qkv, W_out, b_out, c) for c in range(N_CORES)
    ]
    res = run_bass_kernel_spmd(nc, in_maps, list(range(N_CORES)), trace=_trace)

    out = np.empty((B, S, DM), dtype=np.float32)
    for g in range(B):
        acc = res.results[g * CPG]["out"].astype(np.float32)
        for lane in range(1, CPG):
            acc = acc + res.results[g * CPG + lane]["out"]
        out[g] = acc + b_out[None, :].astype(np.float32)

    if _trace:
        kernel.last_exec_time_ns = res.exec_time_ns
        kernel.last_results = res
    return out



# revision 3
# speedup vs baseline: 1.3318x; 1.3318x over previous
"""Multi-head causal self-attention (torch nn.MultiheadAttention semantics)
on 8 Trainium2 NeuronCores.

Problem: x [2, 2048, 1024], 16 heads, head dim 64, fp32, causal, p_drop=0.

Sharding: 2 batch groups x 4-way head tensor-parallel.
  core c: batch b = c // 4, heads [lane*4, lane*4+4) with lane = c % 4.
Each core computes q/k/v projections for its 4 heads, flash-style causal
attention (S^T score layout, no-max softmax), and its partial out-projection.
The host sums the 4 partials per batch and adds b_out.

Performance structure (v2): the whole per-core program is emitted as ONE
software-pipelined instruction stream so the PE sequencer never blocks on a
semaphore. A blocked PE sequencer serializes the next LDWEIGHTS behind the
current matmul AND holds the PE at the 1.2 GHz mid p-state; back-to-back
matmuls hide LDWEIGHTS entirely and ramp the PE to 2.4 GHz after ~3 us
(measured: 230 ns vs 800+ ns per 512-row f32r matmul).

  - Attention runs in 2-head passes (scores psum "sc" 4x[128,512] rotation,
    PV accumulators "pa" 2x[65,512], projections/out-proj "po" 2x[128,512]
    = exactly 8 PSUM banks).
  - q/k/v projections of span sp+1 and out-projection of span sp-1 are
    generators whose matmuls are pulled 2-per-attention-step as PE filler
    between a step's score matmuls and the previous step's PV matmuls,
    covering the exp (Act) latency.
  - exp outputs P^T in bf16 (Act speed is dtype-independent; DVE tri-mask
    gets 2x; PV runs 1 cycle/row at any width, enabling causal trimming of
    the diagonal blocks on both the scores and PV matmuls).
  - softmax denominator rides as a 65th "ones" column of V'; normalization
    is reciprocal_approx_fast + gpsimd partition-broadcast + DVE mul.
"""

import numpy as np
from contextlib import ExitStack

import concourse.bass as bass
import concourse.tile as tile
from concourse import bacc, mybir
from concourse.bass_utils import run_bass_kernel_spmd

F32 = mybir.dt.float32
F32R = mybir.dt.float32r
BF16 = mybir.dt.bfloat16
AF = mybir.ActivationFunctionType

B = 2
S = 2048
DM = 1024
N_HEADS = 16
DH = 64
N_CORES = 8
CPG = 4                    # cores per group (tensor-parallel width)
HPC = N_HEADS // CPG       # 4 heads per core
DQ = HPC * DH              # 256
SPAN = 512
SB = 128
NDM = DM // 128            # 8 x/weight chunks
NSPAN = S // SPAN          # 4
SBS = SPAN // SB           # 4
NSB = S // SB              # 16
NQK = 2 * DQ // 128        # 4 qkT row tiles
NHD = DQ // 128            # 2 OT row tiles
VW = DH + 1                # 65: per-head V plus ones column
OW = 512
NOUT = DM // OW            # 2


def _declare_io(nc):
    t = {}
    t["xT"] = nc.dram_tensor("xT", [DM, S], F32R, kind="ExternalInput").ap()
    t["wqkT"] = nc.dram_tensor("wqkT", [DM, 2 * DQ], F32R, kind="ExternalInput").ap()
    t["wvT"] = nc.dram_tensor("wvT", [DM, DQ], F32R, kind="ExternalInput").ap()
    t["woT"] = nc.dram_tensor("woT", [DQ, DM], F32R, kind="ExternalInput").ap()
    t["bqk"] = nc.dram_tensor("bqk", [2 * DQ, 1], F32, kind="ExternalInput").ap()
    t["bv"] = nc.dram_tensor("bv", [128, DQ], F32, kind="ExternalInput").ap()
    t["out"] = nc.dram_tensor("out", [S, DM], F32, kind="ExternalOutput").ap()
    return t


def _build(ctx: ExitStack, tc: tile.TileContext, io: dict):
    nc = tc.nc

    const = ctx.enter_context(tc.tile_pool(name="const", bufs=1))
    work = ctx.enter_context(tc.tile_pool(name="work", bufs=1))
    psum = ctx.enter_context(tc.tile_pool(name="psum", bufs=1, space="PSUM"))

    # ---- input DMAs: ordered so phase1(span 0) can start ~immediately ----
    bqk = const.tile([128, NQK], F32, name="bqk")
    for obi in range(NQK):
        nc.scalar.dma_start(bqk[:, obi : obi + 1], io["bqk"][obi * 128 : (obi + 1) * 128, :])

    wqk = const.tile([128, NDM * 2 * DQ], F32R, name="wqk")
    xT = const.tile([128, NDM * S], F32R, name="xT")
    for c in range(NDM):
        nc.sync.dma_start(
            wqk[:, c * 2 * DQ : (c + 1) * 2 * DQ], io["wqkT"][c * 128 : (c + 1) * 128, :]
        )
        nc.sync.dma_start(
            xT[:, c * S : c * S + SPAN], io["xT"][c * 128 : (c + 1) * 128, 0:SPAN]
        )
    wv = const.tile([128, NDM * DQ], F32R, name="wv")
    nc.scalar.dma_start(
        wv[:].rearrange("p (c w) -> p c w", w=DQ),
        io["wvT"].rearrange("(c p) w -> p c w", p=128),
    )
    bv = const.tile([128, DQ], F32, name="bv")
    nc.scalar.dma_start(bv[:], io["bv"][:])
    for sp in range(1, NSPAN):
        nc.sync.dma_start(
            xT[:].rearrange("p (c s) -> p c s", s=S)[:, :, sp * SPAN : (sp + 1) * SPAN],
            io["xT"].rearrange("(c p) s -> p c s", p=128)[:, :, sp * SPAN : (sp + 1) * SPAN],
        )
    wo = const.tile([128, NHD * DM], F32R, name="wo")
    nc.scalar.dma_start(
        wo[:].rearrange("p (c w) -> p c w", w=DM),
        io["woT"].rearrange("(c p) w -> p c w", p=128),
    )

    # causal triangle for one diagonal 128x128 sub-block, duplicated for the
    # 2-head strided multiply: tri2[r, j*128+c] = (c - r >= 0)
    tri2 = const.tile([128, 2 * SB], BF16, name="tri2")
    nc.gpsimd.memset(tri2[:], 1.0)
    for half in range(2):
        nc.gpsimd.affine_select(
            out=tri2[:, half * SB : (half + 1) * SB],
            in_=tri2[:, half * SB : (half + 1) * SB],
            compare_op=mybir.AluOpType.is_ge,
            fill=0.0,
            base=0,
            pattern=[[1, SB]],
            channel_multiplier=-1,
        )

    # ---- persistent activations ----
    qkT = [const.tile([128, S], F32R, name=f"qkT{b}") for b in range(NQK)]
    vp = [const.tile([128, HPC * VW], BF16, name=f"vp{sb}") for sb in range(NSB)]
    OT = [const.tile([128, S], F32R, name=f"OT{c}") for c in range(NHD)]

    # ---- filler generators (each yield = one PE matmul emitted) ----
    def gen_phase1(sp):
        for obi in range(NQK):
            pqk = psum.tile([128, SPAN], F32, name=f"pqk_{sp}_{obi}", tag="po", bufs=2)
            for c in range(NDM):
                nc.tensor.matmul(
                    pqk[:],
                    wqk[:, c * 2 * DQ + obi * 128 : c * 2 * DQ + (obi + 1) * 128],
                    xT[:, c * S + sp * SPAN : c * S + (sp + 1) * SPAN],
                    start=(c == 0),
                    stop=(c == NDM - 1),
                    skip_group_check=True,
                )
                if c < NDM - 1:
                    yield
            nc.vector.tensor_scalar_add(
                qkT[obi][:, sp * SPAN : (sp + 1) * SPAN], pqk[:], bqk[:, obi : obi + 1]
            )
            yield

    def gen_phase2(sp):
        for sb in range(sp * SBS, (sp + 1) * SBS):
            pv = psum.tile([128, DQ], F32, name=f"pv_{sb}", tag="po", bufs=2)
            for c in range(NDM):
                nc.tensor.matmul(
                    pv[:],
                    xT[:, c * S + sb * SB : c * S + (sb + 1) * SB],
                    wv[:, c * DQ : (c + 1) * DQ],
                    start=(c == 0),
                    stop=(c == NDM - 1),
                    skip_group_check=True,
                )
                if c < NDM - 1:
                    yield
            vdst = vp[sb][:, :].rearrange("p (h w) -> p h w", w=VW)[:, :, 0:DH]
            nc.vector.tensor_add(
                vdst,
                pv[:].rearrange("p (h d) -> p h d", d=DH),
                bv[:].rearrange("p (h d) -> p h d", d=DH),
            )
            ones = vp[sb][:, DH : HPC * VW : VW]
            nc.vector.memset(ones, 1.0)
            yield

    def gen_outproj(sp):
        for qb in range(sp * SBS, (sp + 1) * SBS):
            ob_t = work.tile([128, DM], F32, name=f"ob_{qb}", tag="ob", bufs=2)
            for nh in range(NOUT):
                pot = psum.tile([128, OW], F32, name=f"pot_{qb}_{nh}", tag="po", bufs=2)
                for c in range(NHD):
                    nc.tensor.matmul(
                        pot[:],
                        OT[c][:, qb * SB : (qb + 1) * SB],
                        wo[:, c * DM + nh * OW : c * DM + (nh + 1) * OW],
                        start=(c == 0),
                        stop=(c == NHD - 1),
                        skip_group_check=True,
                    )
                    yield
                nc.vector.tensor_copy(ob_t[:, nh * OW : (nh + 1) * OW], pot[:])
            nc.sync.dma_start(io["out"][qb * SB : (qb + 1) * SB, :], ob_t[:])
            yield

    queue = []

    def pull(n):
        k = 0
        while queue and k < n:
            try:
                next(queue[0])
                k += 1
            except StopIteration:
                queue.pop(0)

    def drain():
        while queue:
            pull(64)

    # ---- attention for one span, two 2-head passes ----
    def attention_span(sp):
        nsb = (sp + 1) * SBS
        for p in range(2):
            hA = 2 * p
            pts = {}
            pos = {}

            def emit_scores(i):
                sb = i
                d = sb - sp * SBS
                offc = 0 if d < 1 else (128 * d if d < 3 else 256)
                offe = 0 if d < 1 else 128 * d
                pt = work.tile(
                    [128, 2 * SPAN], BF16, name=f"pt_{sp}_{p}_{i}", tag="pt", bufs=3
                )
                pts[i] = (pt, offe)
                for j in range(2):
                    h = hA + j
                    qt = qkT[h // 2]
                    kt = qkT[NQK // 2 + h // 2]
                    qrow = (h % 2) * 64
                    s_t = psum.tile(
                        [128, SPAN], F32, name=f"ps_{sp}_{p}_{i}_{j}", tag="sc", bufs=4
                    )
                    nc.tensor.matmul(
                        s_t[:, offc:SPAN],
                        kt[qrow : qrow + 64, sb * SB : (sb + 1) * SB],
                        qt[qrow : qrow + 64, sp * SPAN + offc : (sp + 1) * SPAN],
                        start=True,
                        stop=True,
                        skip_group_check=True,
                    )
                    nc.scalar.activation(
                        pt[:, j * SPAN + offe : (j + 1) * SPAN],
                        s_t[:, offe:SPAN],
                        AF.Exp,
                        scale=0.125,
                    )
                if d >= 0:
                    ptv = pt[:].rearrange("p (j w) -> p j w", w=SPAN)[
                        :, :, 128 * d : 128 * (d + 1)
                    ]
                    nc.vector.tensor_mul(
                        ptv, ptv, tri2[:].rearrange("p (j w) -> p j w", w=SB)
                    )

            def emit_pv(i):
                sb = i
                d = sb - sp * SBS
                offe = 0 if d < 1 else 128 * d
                pt, _ = pts.pop(i)
                for j in range(2):
                    h = hA + j
                    if i == 0:
                        pos[h] = psum.tile(
                            [VW, SPAN], F32, name=f"po_{sp}_{h}", tag="pa", bufs=2
                        )
                    dst = pos[h][:, offe:SPAN] if offe else pos[h][:]
                    nc.tensor.matmul(
                        dst,
                        vp[sb][:, h * VW : (h + 1) * VW],
                        pt[:, j * SPAN + offe : (j + 1) * SPAN],
                        start=(i == 0),
                        stop=(i == nsb - 1),
                        skip_group_check=True,
                    )

            pull(2)
            for i in range(nsb + 1):
                if i < nsb:
                    emit_scores(i)
                pull(2)
                if i >= 1:
                    emit_pv(i - 1)

            # normalization: out^T rows 0..63 divided by the denominator row
            for j in range(2):
                h = hA + j
                den = work.tile([1, SPAN], F32, name=f"den_{sp}_{h}", tag="den", bufs=2)
                nc.vector.tensor_copy(den[0:1, :], pos[h][VW - 1 : VW, :])
                rden = work.tile([1, SPAN], F32, name=f"rden_{sp}_{h}", tag="rden", bufs=2)
                nc.vector.reciprocal_approx_fast(rden[0:1, :], den[0:1, :])
                recb = work.tile([DH, SPAN], F32, name=f"recb_{sp}_{h}", tag="recb", bufs=2)
                nc.gpsimd.partition_broadcast(recb[:], rden[0:1, :])
                nc.vector.tensor_mul(
                    OT[h // 2][(h % 2) * DH : (h % 2 + 1) * DH, sp * SPAN : (sp + 1) * SPAN],
                    pos[h][0:DH, :],
                    recb[:],
                )

    # ---- main schedule ----
    for _ in gen_phase1(0):
        pass
    for _ in gen_phase2(0):
        pass
    for sp in range(NSPAN):
        if sp + 1 < NSPAN:
            queue.append(gen_phase1(sp + 1))
            queue.append(gen_phase2(sp + 1))
        attention_span(sp)
        drain()
        queue.append(gen_outproj(sp))
    drain()


_NC_CACHE = {}


def _get_compiled():
    if "nc" not in _NC_CACHE:
        nc = bacc.Bacc(
            "TRN2", target_bir_lowering=False, debug=False, num_devices=N_CORES
        )
        io = _declare_io(nc)
        with tile.TileContext(nc) as tc, ExitStack() as ctx:
            _build(ctx, tc, io)
        nc.compile()
        _NC_CACHE["nc"] = nc
    return _NC_CACHE["nc"]


def _prep_core_inputs(x, W_qkv, b_qkv, W_out, b_out, core_id):
    g = core_id // CPG
    lane = core_id % CPG
    h0 = lane * HPC
    r = slice(h0 * DH, (h0 + HPC) * DH)
    Wq = W_qkv[0 * DM : 1 * DM, :][r, :]
    Wk = W_qkv[1 * DM : 2 * DM, :][r, :]
    Wv = W_qkv[2 * DM : 3 * DM, :][r, :]
    bq = b_qkv[0 * DM + h0 * DH : 0 * DM + (h0 + HPC) * DH]
    bk = b_qkv[1 * DM + h0 * DH : 1 * DM + (h0 + HPC) * DH]
    bv_ = b_qkv[2 * DM + h0 * DH : 2 * DM + (h0 + HPC) * DH]
    return {
        "xT": np.ascontiguousarray(x[g].T.astype(np.float32)),
        "wqkT": np.ascontiguousarray(
            np.concatenate([Wq.T, Wk.T], axis=1).astype(np.float32)
        ),
        "wvT": np.ascontiguousarray(Wv.T.astype(np.float32)),
        "woT": np.ascontiguousarray(W_out[:, r].T.astype(np.float32)),
        "bqk": np.concatenate([bq, bk]).reshape(2 * DQ, 1).astype(np.float32),
        "bv": np.ascontiguousarray(
            np.broadcast_to(bv_.reshape(1, DQ), (128, DQ)).astype(np.float32)
        ),
    }


def kernel(x, W_qkv, b_qkv, W_out, b_out, _trace=False):
    x = np.asarray(x)
    W_qkv = np.asarray(W_qkv)
    b_qkv = np.asarray(b_qkv)
    W_out = np.asarray(W_out)
    b_out = np.asarray(b_out)

    nc = _get_compiled()
    in_maps = [
        _prep_core_inputs(x, W_qkv, b_qkv, W_out, b_out, c) for c in range(N_CORES)
    ]
    res = run_bass_kernel_spmd(nc, in_maps, list(range(N_CORES)), trace=_trace)

    out = np.empty((B, S, DM), dtype=np.float32)
    for g in range(B):
        acc = res.results[g * CPG]["out"].astype(np.float32)
        for lane in range(1, CPG):
            acc = acc + res.results[g * CPG + lane]["out"]
        out[g] = acc + b_out[None, :].astype(np.float32)

    if _trace:
        kernel.last_exec_time_ns = res.exec_time_ns
        kernel.last_results = res
    return out


# revision 7
# speedup vs baseline: 1.3862x; 1.0408x over previous
"""Multi-head causal self-attention (torch nn.MultiheadAttention semantics)
on 8 Trainium2 NeuronCores.

Problem: x [2, 2048, 1024], 16 heads, head dim 64, fp32, causal, p_drop=0.

Sharding: 2 batch groups x 4-way head tensor-parallel.
  core c: batch b = c // 4, heads [lane*4, lane*4+4) with lane = c % 4.
Each core computes q/k/v projections for its 4 heads, flash-style causal
attention (S^T score layout, no-max softmax), and its partial out-projection.
The host sums the 4 partials per batch and adds b_out.

Performance structure (v2): the whole per-core program is emitted as ONE
software-pipelined instruction stream so the PE sequencer never blocks on a
semaphore. A blocked PE sequencer serializes the next LDWEIGHTS behind the
current matmul AND holds the PE at the 1.2 GHz mid p-state; back-to-back
matmuls hide LDWEIGHTS entirely and ramp the PE to 2.4 GHz after ~3 us
(measured: 230 ns vs 800+ ns per 512-row f32r matmul).

  - Attention runs in 2-head passes (scores psum "sc" 4x[128,512] rotation,
    PV accumulators "pa" 2x[65,512], projections/out-proj "po" 2x[128,512]
    = exactly 8 PSUM banks).
  - q/k/v projections of span sp+1 and out-projection of span sp-1 are
    generators whose matmuls are pulled 2-per-attention-step as PE filler
    between a step's score matmuls and the previous step's PV matmuls,
    covering the exp (Act) latency.
  - exp outputs P^T in bf16 (Act speed is dtype-independent; DVE tri-mask
    gets 2x; PV runs 1 cycle/row at any width, enabling causal trimming of
    the diagonal blocks on both the scores and PV matmuls).
  - softmax denominator rides as a 65th "ones" column of V'; normalization
    is reciprocal_approx_fast + gpsimd partition-broadcast + DVE mul.
"""

import numpy as np
from contextlib import ExitStack

import concourse.bass as bass
import concourse.tile as tile
from concourse import bacc, mybir
from concourse.bass_utils import run_bass_kernel_spmd

F32 = mybir.dt.float32
F32R = mybir.dt.float32r
BF16 = mybir.dt.bfloat16
AF = mybir.ActivationFunctionType

B = 2
S = 2048
DM = 1024
N_HEADS = 16
DH = 64
N_CORES = 8
CPG = 4                    # cores per group (tensor-parallel width)
HPC = N_HEADS // CPG       # 4 heads per core
DQ = HPC * DH              # 256
SPAN = 512
SB = 128
NDM = DM // 128            # 8 x/weight chunks
NSPAN = S // SPAN          # 4
SBS = SPAN // SB           # 4
NSB = S // SB              # 16
NQK = 2 * DQ // 128        # 4 qkT row tiles
NHD = DQ // 128            # 2 OT row tiles
VW = DH + 1                # 65: per-head V plus ones column
OW = 512
NOUT = DM // OW            # 2


def _declare_io(nc):
    t = {}
    t["xT"] = nc.dram_tensor("xT", [DM, S], F32R, kind="ExternalInput").ap()
    t["wqkT"] = nc.dram_tensor("wqkT", [DM, 2 * DQ], F32R, kind="ExternalInput").ap()
    t["wvT"] = nc.dram_tensor("wvT", [DM, DQ], F32R, kind="ExternalInput").ap()
    t["woT"] = nc.dram_tensor("woT", [DQ, DM], F32R, kind="ExternalInput").ap()
    t["bqk"] = nc.dram_tensor("bqk", [2 * DQ, 1], F32, kind="ExternalInput").ap()
    t["bv"] = nc.dram_tensor("bv", [128, DQ], F32, kind="ExternalInput").ap()
    t["out"] = nc.dram_tensor("out", [S, DM], F32, kind="ExternalOutput").ap()
    return t


def _build(ctx: ExitStack, tc: tile.TileContext, io: dict):
    nc = tc.nc

    const = ctx.enter_context(tc.tile_pool(name="const", bufs=1))
    work = ctx.enter_context(tc.tile_pool(name="work", bufs=1))
    psum = ctx.enter_context(tc.tile_pool(name="psum", bufs=1, space="PSUM"))

    # ---- input DMAs: ordered so phase1(span 0) can start ~immediately ----
    wv = const.tile([128, NDM * DQ], F32R, name="wv")
    nc.scalar.dma_start(
        wv[:].rearrange("p (c w) -> p c w", w=DQ),
        io["wvT"].rearrange("(c p) w -> p c w", p=128),
    )
    bv = const.tile([128, DQ], F32, name="bv")
    nc.scalar.dma_start(bv[:], io["bv"][:])
    bqk = const.tile([128, NQK], F32, name="bqk")
    for obi in range(NQK):
        nc.scalar.dma_start(bqk[:, obi : obi + 1], io["bqk"][obi * 128 : (obi + 1) * 128, :])

    wqk = const.tile([128, NDM * 2 * DQ], F32R, name="wqk")
    xT = const.tile([128, NDM * S], F32R, name="xT")
    for c in range(NDM):
        nc.sync.dma_start(
            wqk[:, c * 2 * DQ : (c + 1) * 2 * DQ], io["wqkT"][c * 128 : (c + 1) * 128, :]
        )
        nc.sync.dma_start(
            xT[:, c * S : c * S + SPAN], io["xT"][c * 128 : (c + 1) * 128, 0:SPAN]
        )
    for sp in range(1, NSPAN):
        nc.sync.dma_start(
            xT[:].rearrange("p (c s) -> p c s", s=S)[:, :, sp * SPAN : (sp + 1) * SPAN],
            io["xT"].rearrange("(c p) s -> p c s", p=128)[:, :, sp * SPAN : (sp + 1) * SPAN],
        )
    wo = const.tile([128, NHD * DM], F32R, name="wo")
    nc.scalar.dma_start(
        wo[:].rearrange("p (c w) -> p c w", w=DM),
        io["woT"].rearrange("(c p) w -> p c w", p=128),
    )

    # causal triangle for one diagonal 128x128 sub-block, duplicated for the
    # 2-head strided multiply: tri2[r, j*128+c] = (c - r >= 0)
    tri2 = const.tile([128, 2 * SB], BF16, name="tri2")
    nc.gpsimd.memset(tri2[:], 1.0)
    for half in range(2):
        nc.gpsimd.affine_select(
            out=tri2[:, half * SB : (half + 1) * SB],
            in_=tri2[:, half * SB : (half + 1) * SB],
            compare_op=mybir.AluOpType.is_ge,
            fill=0.0,
            base=0,
            pattern=[[1, SB]],
            channel_multiplier=-1,
        )

    # ---- persistent activations ----
    qkT = [const.tile([128, S], F32R, name=f"qkT{b}") for b in range(NQK)]
    vp = [const.tile([128, HPC * VW], BF16, name=f"vp{sb}") for sb in range(NSB)]
    OT = [const.tile([128, S], F32R, name=f"OT{c}") for c in range(NHD)]

    # ---- filler generators (each yield = one PE matmul emitted) ----
    def gen_phase1(sp):
        for obi in range(NQK):
            pqk = psum.tile([128, SPAN], F32, name=f"pqk_{sp}_{obi}", tag="po", bufs=2)
            for c in range(NDM):
                nc.tensor.matmul(
                    pqk[:],
                    wqk[:, c * 2 * DQ + obi * 128 : c * 2 * DQ + (obi + 1) * 128],
                    xT[:, c * S + sp * SPAN : c * S + (sp + 1) * SPAN],
                    start=(c == 0),
                    stop=(c == NDM - 1),
                    skip_group_check=True,
                )
                if c < NDM - 1:
                    yield
            nc.vector.tensor_scalar_add(
                qkT[obi][:, sp * SPAN : (sp + 1) * SPAN], pqk[:], bqk[:, obi : obi + 1]
            )
            yield

    def gen_phase2(sp):
        for sb in range(sp * SBS, (sp + 1) * SBS):
            pv = psum.tile([128, DQ], F32, name=f"pv_{sb}", tag="po", bufs=2)
            for c in range(NDM):
                nc.tensor.matmul(
                    pv[:],
                    xT[:, c * S + sb * SB : c * S + (sb + 1) * SB],
                    wv[:, c * DQ : (c + 1) * DQ],
                    start=(c == 0),
                    stop=(c == NDM - 1),
                    skip_group_check=True,
                )
                if c < NDM - 1:
                    yield
            vdst = vp[sb][:, :].rearrange("p (h w) -> p h w", w=VW)[:, :, 0:DH]
            nc.vector.tensor_add(
                vdst,
                pv[:].rearrange("p (h d) -> p h d", d=DH),
                bv[:].rearrange("p (h d) -> p h d", d=DH),
            )
            ones = vp[sb][:, DH : HPC * VW : VW]
            nc.vector.memset(ones, 1.0)
            yield

    def gen_outproj(sp):
        for qb in range(sp * SBS, (sp + 1) * SBS):
            ob_t = work.tile([128, DM], F32, name=f"ob_{qb}", tag="ob", bufs=2)
            for nh in range(NOUT):
                pot = psum.tile([128, OW], F32, name=f"pot_{qb}_{nh}", tag="po", bufs=2)
                for c in range(NHD):
                    nc.tensor.matmul(
                        pot[:],
                        OT[c][:, qb * SB : (qb + 1) * SB],
                        wo[:, c * DM + nh * OW : c * DM + (nh + 1) * OW],
                        start=(c == 0),
                        stop=(c == NHD - 1),
                        skip_group_check=True,
                    )
                    yield
                nc.vector.tensor_copy(ob_t[:, nh * OW : (nh + 1) * OW], pot[:])
            nc.sync.dma_start(io["out"][qb * SB : (qb + 1) * SB, :], ob_t[:])
            yield

    queue = []

    def pull(n):
        k = 0
        while queue and k < n:
            try:
                next(queue[0])
                k += 1
            except StopIteration:
                queue.pop(0)

    def drain():
        while queue:
            pull(64)

    # ---- attention for one span, two 2-head passes ----
    def attention_span(sp):
        nsb = (sp + 1) * SBS
        for p in range(2):
            hA = 2 * p
            pts = {}
            pos = {}

            def emit_scores(i):
                sb = i
                d = sb - sp * SBS
                offc = 0 if d < 1 else (128 * d if d < 3 else 256)
                offe = 0 if d < 1 else 128 * d
                pt = work.tile(
                    [128, 2 * SPAN], BF16, name=f"pt_{sp}_{p}_{i}", tag="pt", bufs=3
                )
                pts[i] = (pt, offe)
                for j in range(2):
                    h = hA + j
                    qt = qkT[h // 2]
                    kt = qkT[NQK // 2 + h // 2]
                    qrow = (h % 2) * 64
                    s_t = psum.tile(
                        [128, SPAN], F32, name=f"ps_{sp}_{p}_{i}_{j}", tag="sc", bufs=4
                    )
                    nc.tensor.matmul(
                        s_t[:, offc:SPAN],
                        kt[qrow : qrow + 64, sb * SB : (sb + 1) * SB],
                        qt[qrow : qrow + 64, sp * SPAN + offc : (sp + 1) * SPAN],
                        start=True,
                        stop=True,
                        skip_group_check=True,
                    )
                    nc.scalar.activation(
                        pt[:, j * SPAN + offe : (j + 1) * SPAN],
                        s_t[:, offe:SPAN],
                        AF.Exp,
                        scale=0.125,
                    )
                if d >= 0:
                    ptv = pt[:].rearrange("p (j w) -> p j w", w=SPAN)[
                        :, :, 128 * d : 128 * (d + 1)
                    ]
                    nc.vector.tensor_mul(
                        ptv, ptv, tri2[:].rearrange("p (j w) -> p j w", w=SB)
                    )

            def emit_pv(i):
                sb = i
                d = sb - sp * SBS
                offe = 0 if d < 1 else 128 * d
                pt, _ = pts.pop(i)
                for j in range(2):
                    h = hA + j
                    if i == 0:
                        pos[h] = psum.tile(
                            [VW, SPAN], F32, name=f"po_{sp}_{h}", tag="pa", bufs=2
                        )
                    dst = pos[h][:, offe:SPAN] if offe else pos[h][:]
                    nc.tensor.matmul(
                        dst,
                        vp[sb][:, h * VW : (h + 1) * VW],
                        pt[:, j * SPAN + offe : (j + 1) * SPAN],
                        start=(i == 0),
                        stop=(i == nsb - 1),
                        skip_group_check=True,
                    )

            pull(4)
            for i in range(nsb + 1):
                if i < nsb:
                    emit_scores(i)
                pull(2)
                if i >= 1:
                    emit_pv(i - 1)

            # normalization: out^T rows 0..63 divided by the denominator row
            for j in range(2):
                h = hA + j
                den = work.tile([1, SPAN], F32, name=f"den_{sp}_{h}", tag="den", bufs=2)
                nc.vector.tensor_copy(den[0:1, :], pos[h][VW - 1 : VW, :])
                rden = work.tile([1, SPAN], F32, name=f"rden_{sp}_{h}", tag="rden", bufs=2)
                nc.vector.reciprocal_approx_fast(rden[0:1, :], den[0:1, :])
                recb = work.tile([DH, SPAN], F32, name=f"recb_{sp}_{h}", tag="recb", bufs=2)
                nc.gpsimd.partition_broadcast(recb[:], rden[0:1, :])
                nc.vector.tensor_mul(
                    OT[h // 2][(h % 2) * DH : (h % 2 + 1) * DH, sp * SPAN : (sp + 1) * SPAN],
                    pos[h][0:DH, :],
                    recb[:],
                )

    # ---- span-0 projections, contraction-chunk OUTER so each x/w chunk is
    # consumed as its DMA lands (the attention "sc" and "po" psum banks are
    # free this early, giving 4 qk + 2x2 v accumulators) ----
    pqk0 = [psum.tile([128, SPAN], F32, name=f"pqk0_{obi}", tag="sc", bufs=4) for obi in range(NQK)]
    pv0 = [
        psum.tile([128, DQ], F32, name=f"pv0_{sb}", tag=("po" if sb < 2 else "pa"), bufs=2)
        for sb in range(SBS)
    ]
    for c in range(NDM):
        for obi in range(NQK):
            nc.tensor.matmul(
                pqk0[obi][:],
                wqk[:, c * 2 * DQ + obi * 128 : c * 2 * DQ + (obi + 1) * 128],
                xT[:, c * S : c * S + SPAN],
                start=(c == 0),
                stop=(c == NDM - 1),
                skip_group_check=True,
            )
        for sb in range(SBS):
            nc.tensor.matmul(
                pv0[sb][:],
                xT[:, c * S + sb * SB : c * S + (sb + 1) * SB],
                wv[:, c * DQ : (c + 1) * DQ],
                start=(c == 0),
                stop=(c == NDM - 1),
                skip_group_check=True,
            )
    for obi in range(NQK):
        nc.vector.tensor_scalar_add(
            qkT[obi][:, 0:SPAN], pqk0[obi][:], bqk[:, obi : obi + 1]
        )
    for sb in range(SBS):
        vdst = vp[sb][:, :].rearrange("p (h w) -> p h w", w=VW)[:, :, 0:DH]
        nc.vector.tensor_add(
            vdst,
            pv0[sb][:].rearrange("p (h d) -> p h d", d=DH),
            bv[:].rearrange("p (h d) -> p h d", d=DH),
        )
        nc.vector.memset(vp[sb][:, DH : HPC * VW : VW], 1.0)
    for sp in range(NSPAN):
        if sp + 1 < NSPAN:
            queue.append(gen_phase1(sp + 1))
            queue.append(gen_phase2(sp + 1))
        attention_span(sp)
        drain()
        queue.append(gen_outproj(sp))
    drain()


_NC_CACHE = {}


def _get_compiled():
    if "nc" not in _NC_CACHE:
        nc = bacc.Bacc(
            "TRN2", target_bir_lowering=False, debug=False, num_devices=N_CORES
        )
        io = _declare_io(nc)
        with tile.TileContext(nc) as tc, ExitStack() as ctx:
            _build(ctx, tc, io)
        nc.compile()
        _NC_CACHE["nc"] = nc
    return _NC_CACHE["nc"]


def _prep_core_inputs(x, W_qkv, b_qkv, W_out, b_out, core_id):
    g = core_id // CPG
    lane = core_id % CPG
    h0 = lane * HPC
    r = slice(h0 * DH, (h0 + HPC) * DH)
    Wq = W_qkv[0 * DM : 1 * DM, :][r, :]
    Wk = W_qkv[1 * DM : 2 * DM, :][r, :]
    Wv = W_qkv[2 * DM : 3 * DM, :][r, :]
    bq = b_qkv[0 * DM + h0 * DH : 0 * DM + (h0 + HPC) * DH]
    bk = b_qkv[1 * DM + h0 * DH : 1 * DM + (h0 + HPC) * DH]
    bv_ = b_qkv[2 * DM + h0 * DH : 2 * DM + (h0 + HPC) * DH]
    return {
        "xT": np.ascontiguousarray(x[g].T.astype(np.float32)),
        "wqkT": np.ascontiguousarray(
            np.concatenate([Wq.T, Wk.T], axis=1).astype(np.float32)
        ),
        "wvT": np.ascontiguousarray(Wv.T.astype(np.float32)),
        "woT": np.ascontiguousarray(W_out[:, r].T.astype(np.float32)),
        "bqk": np.concatenate([bq, bk]).reshape(2 * DQ, 1).astype(np.float32),
        "bv": np.ascontiguousarray(
            np.broadcast_to(bv_.reshape(1, DQ), (128, DQ)).astype(np.float32)
        ),
    }


def kernel(x, W_qkv, b_qkv, W_out, b_out, _trace=False):
    x = np.asarray(x)
    W_qkv = np.asarray(W_qkv)
    b_qkv = np.asarray(b_qkv)
    W_out = np.asarray(W_out)
    b_out = np.asarray(b_out)

    nc = _get_compiled()
    in_maps = [
        _prep_core_inputs(x, W_qkv, b_qkv, W_out, b_out, c) for c in range(N_CORES)
    ]
    res = run_bass_kernel_spmd(nc, in_maps, list(range(N_CORES)), trace=_trace)

    out = np.empty((B, S, DM), dtype=np.float32)
    for g in range(B):
        acc = res.results[g * CPG]["out"].astype(np.float32)
        for lane in range(1, CPG):
            acc = acc + res.results[g * CPG + lane]["out"]
        out[g] = acc + b_out[None, :].astype(np.float32)

    if _trace:
        kernel.last_exec_time_ns = res.exec_time_ns
        kernel.last_results = res
    return out


# revision 17
# speedup vs baseline: 1.6252x; 1.1725x over previous
"""Multi-head causal self-attention (torch nn.MultiheadAttention semantics)
on 8 Trainium2 NeuronCores.

Problem: x [2, 2048, 1024], 16 heads, head dim 64, fp32, causal, p_drop=0.

Sharding: 2 batch groups x 4-way head tensor-parallel.
  core c: batch b = c // 4, heads [lane*4, lane*4+4) with lane = c % 4.
Each core computes q/k/v projections for its 4 heads, flash-style causal
attention (S^T score layout, no-max softmax), and its partial out-projection.
The host sums the 4 partials per batch and adds b_out.

Performance structure (v2): the whole per-core program is emitted as ONE
software-pipelined instruction stream so the PE sequencer never blocks on a
semaphore. A blocked PE sequencer serializes the next LDWEIGHTS behind the
current matmul AND holds the PE at the 1.2 GHz mid p-state; back-to-back
matmuls hide LDWEIGHTS entirely and ramp the PE to 2.4 GHz after ~3 us
(measured: 230 ns vs 800+ ns per 512-row f32r matmul).

  - Attention runs in 2-head passes (scores psum "sc" 4x[128,512] rotation,
    PV accumulators "pa" 2x[65,512], projections/out-proj "po" 2x[128,512]
    = exactly 8 PSUM banks).
  - q/k/v projections of span sp+1 and out-projection of span sp-1 are
    generators whose matmuls are pulled 2-per-attention-step as PE filler
    between a step's score matmuls and the previous step's PV matmuls,
    covering the exp (Act) latency.
  - exp outputs P^T in bf16 (Act speed is dtype-independent; DVE tri-mask
    gets 2x; PV runs 1 cycle/row at any width, enabling causal trimming of
    the diagonal blocks on both the scores and PV matmuls).
  - softmax denominator rides as a 65th "ones" column of V'; normalization
    is reciprocal_approx_fast + gpsimd partition-broadcast + DVE mul.
"""

import ml_dtypes
import numpy as np
from contextlib import ExitStack

import concourse.bass as bass
import concourse.tile as tile
from concourse import bacc, mybir
from concourse.bass_utils import run_bass_kernel_spmd

F32 = mybir.dt.float32
F32R = mybir.dt.float32r
BF16 = mybir.dt.bfloat16
AF = mybir.ActivationFunctionType

B = 2
S = 2048
DM = 1024
N_HEADS = 16
DH = 64
N_CORES = 8
CPG = 4                    # cores per group (tensor-parallel width)
HPC = N_HEADS // CPG       # 4 heads per core
DQ = HPC * DH              # 256
SPAN = 512
SB = 128
NDM = DM // 128            # 8 x/weight chunks
NSPAN = S // SPAN          # 4
SBS = SPAN // SB           # 4
NSB = S // SB              # 16
NQK = 2 * DQ // 128        # 4 qkT row tiles
NHD = DQ // 128            # 2 OT row tiles
VW = DH + 1                # 65: per-head V plus ones column
OW = 512
NOUT = DM // OW            # 2


def _declare_io(nc):
    t = {}
    t["xT"] = nc.dram_tensor("xT", [DM, S], BF16, kind="ExternalInput").ap()
    t["wqkT"] = nc.dram_tensor("wqkT", [DM, 2 * DQ], BF16, kind="ExternalInput").ap()
    t["wvT"] = nc.dram_tensor("wvT", [DM, DQ], BF16, kind="ExternalInput").ap()
    t["woT"] = nc.dram_tensor("woT", [DQ, DM], F32R, kind="ExternalInput").ap()
    t["bqk"] = nc.dram_tensor("bqk", [2 * DQ, 1], F32, kind="ExternalInput").ap()
    t["bv"] = nc.dram_tensor("bv", [128, DQ], F32, kind="ExternalInput").ap()
    t["out"] = nc.dram_tensor("out", [S, DM], F32, kind="ExternalOutput").ap()
    return t


def _build(ctx: ExitStack, tc: tile.TileContext, io: dict):
    nc = tc.nc

    const = ctx.enter_context(tc.tile_pool(name="const", bufs=1))
    work = ctx.enter_context(tc.tile_pool(name="work", bufs=1))
    psum = ctx.enter_context(tc.tile_pool(name="psum", bufs=1, space="PSUM"))

    # ---- input DMAs: ordered so phase1(span 0) can start ~immediately ----
    wv = const.tile([128, NDM * DQ], BF16, name="wv")
    nc.scalar.dma_start(
        wv[:].rearrange("p (c w) -> p c w", w=DQ),
        io["wvT"].rearrange("(c p) w -> p c w", p=128),
    )
    bv = const.tile([128, DQ], F32, name="bv")
    nc.scalar.dma_start(bv[:], io["bv"][:])
    bqk = const.tile([128, NQK], F32, name="bqk")
    for obi in range(NQK):
        nc.scalar.dma_start(bqk[:, obi : obi + 1], io["bqk"][obi * 128 : (obi + 1) * 128, :])

    wqk = const.tile([128, NDM * 2 * DQ], BF16, name="wqk")
    xT = const.tile([128, NDM * S], BF16, name="xT")
    for c in range(NDM):
        nc.sync.dma_start(
            wqk[:, c * 2 * DQ : (c + 1) * 2 * DQ], io["wqkT"][c * 128 : (c + 1) * 128, :]
        )
        nc.sync.dma_start(
            xT[:, c * S : c * S + SPAN], io["xT"][c * 128 : (c + 1) * 128, 0:SPAN]
        )
    for sp in range(1, NSPAN):
        nc.sync.dma_start(
            xT[:].rearrange("p (c s) -> p c s", s=S)[:, :, sp * SPAN : (sp + 1) * SPAN],
            io["xT"].rearrange("(c p) s -> p c s", p=128)[:, :, sp * SPAN : (sp + 1) * SPAN],
        )
    wo = const.tile([128, NHD * DM], F32R, name="wo")
    nc.scalar.dma_start(
        wo[:].rearrange("p (c w) -> p c w", w=DM),
        io["woT"].rearrange("(c p) w -> p c w", p=128),
    )

    # causal triangle for one diagonal 128x128 sub-block, duplicated for the
    # 2-head strided multiply: tri2[r, j*128+c] = (c - r >= 0)
    tri2 = const.tile([128, 2 * SB], BF16, name="tri2")
    nc.gpsimd.memset(tri2[:], 1.0)
    for half in range(2):
        nc.gpsimd.affine_select(
            out=tri2[:, half * SB : (half + 1) * SB],
            in_=tri2[:, half * SB : (half + 1) * SB],
            compare_op=mybir.AluOpType.is_ge,
            fill=0.0,
            base=0,
            pattern=[[1, SB]],
            channel_multiplier=-1,
        )

    # ---- persistent activations ----
    qkT = [const.tile([128, S], F32R, name=f"qkT{b}") for b in range(NQK)]
    vp = [const.tile([128, HPC * VW], BF16, name=f"vp{sb}") for sb in range(NSB)]
    OT = [const.tile([128, S], F32R, name=f"OT{c}") for c in range(NHD)]

    # ---- filler generators (each yield = one PE matmul emitted) ----
    def gen_phase1(sp):
        for obi in range(NQK):
            pqk = psum.tile([128, SPAN], F32, name=f"pqk_{sp}_{obi}", tag="po", bufs=2)
            for c in range(NDM):
                nc.tensor.matmul(
                    pqk[:],
                    wqk[:, c * 2 * DQ + obi * 128 : c * 2 * DQ + (obi + 1) * 128],
                    xT[:, c * S + sp * SPAN : c * S + (sp + 1) * SPAN],
                    start=(c == 0),
                    stop=(c == NDM - 1),
                    skip_group_check=True,
                )
                if c < NDM - 1:
                    yield
            nc.vector.tensor_scalar_add(
                qkT[obi][:, sp * SPAN : (sp + 1) * SPAN], pqk[:], bqk[:, obi : obi + 1]
            )
            yield

    def gen_phase2(sp):
        for sb in range(sp * SBS, (sp + 1) * SBS):
            pv = psum.tile([128, DQ], F32, name=f"pv_{sb}", tag="po", bufs=2)
            for c in range(NDM):
                nc.tensor.matmul(
                    pv[:],
                    xT[:, c * S + sb * SB : c * S + (sb + 1) * SB],
                    wv[:, c * DQ : (c + 1) * DQ],
                    start=(c == 0),
                    stop=(c == NDM - 1),
                    skip_group_check=True,
                )
                if c < NDM - 1:
                    yield
            vdst = vp[sb][:, :].rearrange("p (h w) -> p h w", w=VW)[:, :, 0:DH]
            nc.vector.tensor_add(
                vdst,
                pv[:].rearrange("p (h d) -> p h d", d=DH),
                bv[:].rearrange("p (h d) -> p h d", d=DH),
            )
            ones = vp[sb][:, DH : HPC * VW : VW]
            nc.vector.memset(ones, 1.0)
            yield

    def gen_outproj(sp):
        for qb in range(sp * SBS, (sp + 1) * SBS):
            ob_t = work.tile([128, DM], F32, name=f"ob_{qb}", tag="ob", bufs=2)
            for nh in range(NOUT):
                pot = psum.tile([128, OW], F32, name=f"pot_{qb}_{nh}", tag="po", bufs=2)
                for c in range(NHD):
                    nc.tensor.matmul(
                        pot[:],
                        OT[c][:, qb * SB : (qb + 1) * SB],
                        wo[:, c * DM + nh * OW : c * DM + (nh + 1) * OW],
                        start=(c == 0),
                        stop=(c == NHD - 1),
                        skip_group_check=True,
                    )
                    yield
                nc.vector.tensor_copy(ob_t[:, nh * OW : (nh + 1) * OW], pot[:])
            nc.sync.dma_start(io["out"][qb * SB : (qb + 1) * SB, :], ob_t[:])
            yield

    # Two-level filler queue: "prio" holds next-span projections (must finish
    # before that span's attention), "bulk" holds out-projections (no deadline
    # until the end — saved to keep the PE fed through span 3's long passes).
    prio = []
    bulk = []

    def pull(n):
        k = 0
        while k < n:
            q = prio if prio else bulk
            if not q:
                return
            try:
                next(q[0])
                k += 1
            except StopIteration:
                q.pop(0)

    def drain_prio():
        while prio:
            pull(64)

    def drain_all():
        while prio or bulk:
            pull(64)

    # ---- attention for one span, two 2-head passes ----
    def attention_span(sp):
        nsb = (sp + 1) * SBS
        for p in range(2):
            hA = 2 * p
            pts = {}
            pos = {}

            def emit_scores(i):
                sb = i
                d = sb - sp * SBS
                offc = 0 if d < 1 else (128 * d if d < 3 else 256)
                offe = 0 if d < 1 else 128 * d
                pt = work.tile(
                    [128, 2 * SPAN], BF16, name=f"pt_{sp}_{p}_{i}", tag="pt", bufs=3
                )
                pts[i] = (pt, offe)
                s_t = psum.tile(
                    [128, 2 * SPAN], F32, name=f"ps_{sp}_{p}_{i}", tag="sc", bufs=2
                )
                for j in range(2):
                    h = hA + j
                    qt = qkT[h // 2]
                    kt = qkT[NQK // 2 + h // 2]
                    qrow = (h % 2) * 64
                    nc.tensor.matmul(
                        s_t[:, j * SPAN + offc : (j + 1) * SPAN],
                        kt[qrow : qrow + 64, sb * SB : (sb + 1) * SB],
                        qt[qrow : qrow + 64, sp * SPAN + offc : (sp + 1) * SPAN],
                        start=True,
                        stop=True,
                        skip_group_check=True,
                    )
                # one exp covers both heads' (trimmed) score blocks
                nc.scalar.activation(
                    pt[:].rearrange("p (j w) -> p j w", w=SPAN)[:, :, offe:SPAN],
                    s_t[:].rearrange("p (j w) -> p j w", w=SPAN)[:, :, offe:SPAN],
                    AF.Exp,
                    scale=0.125,
                )
                if d >= 0:
                    ptv = pt[:].rearrange("p (j w) -> p j w", w=SPAN)[
                        :, :, 128 * d : 128 * (d + 1)
                    ]
                    nc.vector.tensor_mul(
                        ptv, ptv, tri2[:].rearrange("p (j w) -> p j w", w=SB)
                    )

            def emit_pv(i):
                sb = i
                d = sb - sp * SBS
                offe = 0 if d < 1 else 128 * d
                pt, _ = pts.pop(i)
                for j in range(2):
                    h = hA + j
                    if i == 0:
                        pos[h] = psum.tile(
                            [VW, SPAN], F32, name=f"po_{sp}_{h}", tag="pa", bufs=2
                        )
                    dst = pos[h][:, offe:SPAN] if offe else pos[h][:]
                    nc.tensor.matmul(
                        dst,
                        vp[sb][:, h * VW : (h + 1) * VW],
                        pt[:, j * SPAN + offe : (j + 1) * SPAN],
                        start=(i == 0),
                        stop=(i == nsb - 1),
                        skip_group_check=True,
                    )

            pull(4)
            for i in range(nsb + 1):
                if i < nsb:
                    emit_scores(i)
                pull(2)
                if i >= 1:
                    emit_pv(i - 1)

            # normalization: out^T rows 0..63 divided by the denominator row
            for j in range(2):
                h = hA + j
                den = work.tile([1, SPAN], F32, name=f"den_{sp}_{h}", tag="den", bufs=2)
                nc.vector.tensor_copy(den[0:1, :], pos[h][VW - 1 : VW, :])
                rden = work.tile([1, SPAN], F32, name=f"rden_{sp}_{h}", tag="rden", bufs=2)
                nc.vector.reciprocal_approx_fast(rden[0:1, :], den[0:1, :])
                recb = work.tile([DH, SPAN], F32, name=f"recb_{sp}_{h}", tag="recb", bufs=2)
                nc.gpsimd.partition_broadcast(recb[:], rden[0:1, :])
                nc.vector.tensor_mul(
                    OT[h // 2][(h % 2) * DH : (h % 2 + 1) * DH, sp * SPAN : (sp + 1) * SPAN],
                    pos[h][0:DH, :],
                    recb[:],
                )

    # ---- span-0 projections, contraction-chunk OUTER so each x/w chunk is
    # consumed as its DMA lands (the attention "sc" and "po" psum banks are
    # free this early, giving 4 qk + 2x2 v accumulators) ----
    # pqk0: ob pairs packed into 2-bank "sc" tiles (independent accumulation
    # groups may share a TILE but never a BANK — halves sit in separate banks)
    pqk0 = [psum.tile([128, 2 * SPAN], F32, name=f"pqk0_{i}", tag="sc", bufs=2) for i in range(2)]
    pv0 = [
        psum.tile([128, DQ], F32, name=f"pv0_{sb}", tag=("po" if sb < 2 else "pa"), bufs=2)
        for sb in range(SBS)
    ]
    for c in range(NDM):
        for obi in range(NQK):
            nc.tensor.matmul(
                pqk0[obi // 2][:, (obi % 2) * SPAN : (obi % 2 + 1) * SPAN],
                wqk[:, c * 2 * DQ + obi * 128 : c * 2 * DQ + (obi + 1) * 128],
                xT[:, c * S : c * S + SPAN],
                start=(c == 0),
                stop=(c == NDM - 1),
                skip_group_check=True,
            )
        for sb in range(SBS):
            nc.tensor.matmul(
                pv0[sb][:],
                xT[:, c * S + sb * SB : c * S + (sb + 1) * SB],
                wv[:, c * DQ : (c + 1) * DQ],
                start=(c == 0),
                stop=(c == NDM - 1),
                skip_group_check=True,
            )
    for obi in (0, 2, 1, 3):  # pass-A tiles (q heads 0/1, k heads 0/1) first
        nc.vector.tensor_scalar_add(
            qkT[obi][:, 0:SPAN],
            pqk0[obi // 2][:, (obi % 2) * SPAN : (obi % 2 + 1) * SPAN],
            bqk[:, obi : obi + 1],
        )
    for sb in range(SBS):
        vdst = vp[sb][:, :].rearrange("p (h w) -> p h w", w=VW)[:, :, 0:DH]
        nc.vector.tensor_add(
            vdst,
            pv0[sb][:].rearrange("p (h d) -> p h d", d=DH),
            bv[:].rearrange("p (h d) -> p h d", d=DH),
        )
        nc.vector.memset(vp[sb][:, DH : HPC * VW : VW], 1.0)
    for sp in range(NSPAN):
        if sp + 1 < NSPAN:
            prio.append(gen_phase1(sp + 1))
            prio.append(gen_phase2(sp + 1))
        attention_span(sp)
        drain_prio()
        if sp < 2:
            bulk.append(gen_outproj(sp))
    # op(2) held back: its 16 ready-to-run matmuls keep the PE hot across the
    # final normalization chain; op(3) follows right behind.
    bulk.append(gen_outproj(2))
    bulk.append(gen_outproj(3))
    drain_all()


_NC_CACHE = {}


def _get_compiled():
    if "nc" not in _NC_CACHE:
        nc = bacc.Bacc(
            "TRN2", target_bir_lowering=False, debug=False, num_devices=N_CORES
        )
        io = _declare_io(nc)
        with tile.TileContext(nc) as tc, ExitStack() as ctx:
            _build(ctx, tc, io)
        nc.compile()
        _NC_CACHE["nc"] = nc
    return _NC_CACHE["nc"]


def _prep_core_inputs(x, W_qkv, b_qkv, W_out, b_out, core_id):
    g = core_id // CPG
    lane = core_id % CPG
    h0 = lane * HPC
    r = slice(h0 * DH, (h0 + HPC) * DH)
    Wq = W_qkv[0 * DM : 1 * DM, :][r, :]
    Wk = W_qkv[1 * DM : 2 * DM, :][r, :]
    Wv = W_qkv[2 * DM : 3 * DM, :][r, :]
    bq = b_qkv[0 * DM + h0 * DH : 0 * DM + (h0 + HPC) * DH]
    bk = b_qkv[1 * DM + h0 * DH : 1 * DM + (h0 + HPC) * DH]
    bv_ = b_qkv[2 * DM + h0 * DH : 2 * DM + (h0 + HPC) * DH]
    return {
        "xT": np.ascontiguousarray(x[g].T.astype(ml_dtypes.bfloat16)),
        "wqkT": np.ascontiguousarray(
            np.concatenate([Wq.T, Wk.T], axis=1).astype(ml_dtypes.bfloat16)
        ),
        "wvT": np.ascontiguousarray(Wv.T.astype(ml_dtypes.bfloat16)),
        "woT": np.ascontiguousarray(W_out[:, r].T.astype(np.float32)),
        "bqk": np.concatenate([bq, bk]).reshape(2 * DQ, 1).astype(np.float32),
        "bv": np.ascontiguousarray(
            np.broadcast_to(bv_.reshape(1, DQ), (128, DQ)).astype(np.float32)
        ),
    }


def kernel(x, W_qkv, b_qkv, W_out, b_out, _trace=False):
    x = np.asarray(x)
    W_qkv = np.asarray(W_qkv)
    b_qkv = np.asarray(b_qkv)
    W_out = np.asarray(W_out)
    b_out = np.asarray(b_out)

    nc = _get_compiled()
    in_maps = [
        _prep_core_inputs(x, W_qkv, b_qkv, W_out, b_out, c) for c in range(N_CORES)
    ]
    res = run_bass_kernel_spmd(nc, in_maps, list(range(N_CORES)), trace=_trace)

    out = np.empty((B, S, DM), dtype=np.float32)
    for g in range(B):
        acc = res.results[g * CPG]["out"].astype(np.float32)
        for lane in range(1, CPG):
            acc = acc + res.results[g * CPG + lane]["out"]
        out[g] = acc + b_out[None, :].astype(np.float32)

    if _trace:
        kernel.last_exec_time_ns = res.exec_time_ns
        kernel.last_results = res
    return out


# revision 28
# speedup vs baseline: 1.6715x; 1.0285x over previous
"""Multi-head causal self-attention (torch nn.MultiheadAttention semantics)
on 8 Trainium2 NeuronCores.

Problem: x [2, 2048, 1024], 16 heads, head dim 64, fp32, causal, p_drop=0.

Sharding: 2 batch groups x 4-way head tensor-parallel.
  core c: batch b = c // 4, heads [lane*4, lane*4+4) with lane = c % 4.
Each core computes q/k/v projections for its 4 heads, flash-style causal
attention (S^T score layout, no-max softmax), and its partial out-projection.
The host sums the 4 partials per batch and adds b_out.

Performance structure (v2): the whole per-core program is emitted as ONE
software-pipelined instruction stream so the PE sequencer never blocks on a
semaphore. A blocked PE sequencer serializes the next LDWEIGHTS behind the
current matmul AND holds the PE at the 1.2 GHz mid p-state; back-to-back
matmuls hide LDWEIGHTS entirely and ramp the PE to 2.4 GHz after ~3 us
(measured: 230 ns vs 800+ ns per 512-row f32r matmul).

  - Attention runs in 2-head passes (scores psum "sc" 4x[128,512] rotation,
    PV accumulators "pa" 2x[65,512], projections/out-proj "po" 2x[128,512]
    = exactly 8 PSUM banks).
  - q/k/v projections of span sp+1 and out-projection of span sp-1 are
    generators whose matmuls are pulled 2-per-attention-step as PE filler
    between a step's score matmuls and the previous step's PV matmuls,
    covering the exp (Act) latency.
  - exp outputs P^T in bf16 (Act speed is dtype-independent; DVE tri-mask
    gets 2x; PV runs 1 cycle/row at any width, enabling causal trimming of
    the diagonal blocks on both the scores and PV matmuls).
  - softmax denominator rides as a 65th "ones" column of V'; normalization
    is reciprocal_approx_fast + gpsimd partition-broadcast + DVE mul.
"""

import ml_dtypes
import numpy as np
from contextlib import ExitStack

import concourse.bass as bass
import concourse.tile as tile
from concourse import bacc, mybir
from concourse.bass_utils import run_bass_kernel_spmd

F32 = mybir.dt.float32
F32R = mybir.dt.float32r
BF16 = mybir.dt.bfloat16
AF = mybir.ActivationFunctionType

B = 2
S = 2048
DM = 1024
N_HEADS = 16
DH = 64
N_CORES = 8
CPG = 4                    # cores per group (tensor-parallel width)
HPC = N_HEADS // CPG       # 4 heads per core
DQ = HPC * DH              # 256
SPAN = 512
SB = 128
NDM = DM // 128            # 8 x/weight chunks
NSPAN = S // SPAN          # 4
SBS = SPAN // SB           # 4
NSB = S // SB              # 16
NQK = 2 * DQ // 128        # 4 qkT row tiles
NHD = DQ // 128            # 2 OT row tiles
VW = DH + 1                # 65: per-head V plus ones column
OW = 512
NOUT = DM // OW            # 2


def _declare_io(nc):
    t = {}
    t["xT"] = nc.dram_tensor("xT", [DM, S], BF16, kind="ExternalInput").ap()
    t["wqkT"] = nc.dram_tensor("wqkT", [DM, 2 * DQ], BF16, kind="ExternalInput").ap()
    t["wvT"] = nc.dram_tensor("wvT", [DM, DQ], BF16, kind="ExternalInput").ap()
    t["woT"] = nc.dram_tensor("woT", [DQ, DM], F32R, kind="ExternalInput").ap()
    t["bqk"] = nc.dram_tensor("bqk", [2 * DQ, 1], F32, kind="ExternalInput").ap()
    t["bv"] = nc.dram_tensor("bv", [128, DQ], F32, kind="ExternalInput").ap()
    t["out"] = nc.dram_tensor("out", [S, DM], F32, kind="ExternalOutput").ap()
    return t


def _build(ctx: ExitStack, tc: tile.TileContext, io: dict):
    nc = tc.nc

    const = ctx.enter_context(tc.tile_pool(name="const", bufs=1))
    work = ctx.enter_context(tc.tile_pool(name="work", bufs=1))
    psum = ctx.enter_context(tc.tile_pool(name="psum", bufs=1, space="PSUM"))

    # ---- input DMAs: ordered so phase1(span 0) can start ~immediately ----
    wv = const.tile([128, NDM * DQ], BF16, name="wv")
    nc.scalar.dma_start(
        wv[:].rearrange("p (c w) -> p c w", w=DQ),
        io["wvT"].rearrange("(c p) w -> p c w", p=128),
    )
    bv = const.tile([128, DQ], F32, name="bv")
    nc.scalar.dma_start(bv[:], io["bv"][:])
    bqk = const.tile([128, NQK], F32, name="bqk")
    for obi in range(NQK):
        nc.scalar.dma_start(bqk[:, obi : obi + 1], io["bqk"][obi * 128 : (obi + 1) * 128, :])

    wqk = const.tile([128, NDM * 2 * DQ], BF16, name="wqk")
    xT = const.tile([128, NDM * S], BF16, name="xT")
    # half-batched loads: chunks 0-3 then 4-7, x-span0 ahead of wqk
    for lo in (0, 4):
        nc.sync.dma_start(
            xT[:].rearrange("p (c s) -> p c s", s=S)[:, lo : lo + 4, 0:SPAN],
            io["xT"].rearrange("(c p) s -> p c s", p=128)[:, lo : lo + 4, 0:SPAN],
        )
        nc.sync.dma_start(
            wqk[:].rearrange("p (c w) -> p c w", w=2 * DQ)[:, lo : lo + 4, :],
            io["wqkT"].rearrange("(c p) w -> p c w", p=128)[:, lo : lo + 4, :],
        )
    for sp in range(1, NSPAN):
        nc.sync.dma_start(
            xT[:].rearrange("p (c s) -> p c s", s=S)[:, :, sp * SPAN : (sp + 1) * SPAN],
            io["xT"].rearrange("(c p) s -> p c s", p=128)[:, :, sp * SPAN : (sp + 1) * SPAN],
        )
    wo = const.tile([128, NHD * DM], F32R, name="wo")
    nc.scalar.dma_start(
        wo[:].rearrange("p (c w) -> p c w", w=DM),
        io["woT"].rearrange("(c p) w -> p c w", p=128),
    )

    # causal triangle for one diagonal 128x128 sub-block, duplicated for the
    # 2-head strided multiply: tri2[r, j*128+c] = (c - r >= 0)
    tri2 = const.tile([128, 2 * SB], BF16, name="tri2")
    nc.gpsimd.memset(tri2[:], 1.0)
    for half in range(2):
        nc.gpsimd.affine_select(
            out=tri2[:, half * SB : (half + 1) * SB],
            in_=tri2[:, half * SB : (half + 1) * SB],
            compare_op=mybir.AluOpType.is_ge,
            fill=0.0,
            base=0,
            pattern=[[1, SB]],
            channel_multiplier=-1,
        )

    # ---- persistent activations ----
    qkT = [const.tile([128, S], F32R, name=f"qkT{b}") for b in range(NQK)]
    vp = [const.tile([128, HPC * VW], BF16, name=f"vp{sb}") for sb in range(NSB)]
    OT = [const.tile([128, S], F32R, name=f"OT{c}") for c in range(NHD)]

    vp_ready = [False] * NSB
    p1b = {}  # span -> generator for that span's pass-B qk projections

    # ---- filler generators (each yield = one PE matmul emitted) ----
    def gen_phase1(sp, obs):
        for obi in obs:
            pqk = psum.tile([128, SPAN], F32, name=f"pqk_{sp}_{obi}", tag="po", bufs=2)
            for c in range(NDM):
                nc.tensor.matmul(
                    pqk[:],
                    wqk[:, c * 2 * DQ + obi * 128 : c * 2 * DQ + (obi + 1) * 128],
                    xT[:, c * S + sp * SPAN : c * S + (sp + 1) * SPAN],
                    start=(c == 0),
                    stop=(c == NDM - 1),
                    skip_group_check=True,
                )
                if c < NDM - 1:
                    yield
            nc.vector.tensor_scalar_add(
                qkT[obi][:, sp * SPAN : (sp + 1) * SPAN], pqk[:], bqk[:, obi : obi + 1]
            )
            yield

    def gen_phase2(sp):
        for sb in range(sp * SBS, (sp + 1) * SBS):
            pv = psum.tile([128, DQ], F32, name=f"pv_{sb}", tag="po", bufs=2)
            for c in range(NDM):
                nc.tensor.matmul(
                    pv[:],
                    xT[:, c * S + sb * SB : c * S + (sb + 1) * SB],
                    wv[:, c * DQ : (c + 1) * DQ],
                    start=(c == 0),
                    stop=(c == NDM - 1),
                    skip_group_check=True,
                )
                if c < NDM - 1:
                    yield
            vdst = vp[sb][:, :].rearrange("p (h w) -> p h w", w=VW)[:, :, 0:DH]
            nc.vector.tensor_add(
                vdst,
                pv[:].rearrange("p (h d) -> p h d", d=DH),
                bv[:].rearrange("p (h d) -> p h d", d=DH),
            )
            ones = vp[sb][:, DH : HPC * VW : VW]
            nc.vector.memset(ones, 1.0)
            vp_ready[sb] = True
            yield

    def gen_outproj(sp):
        for qb in range(sp * SBS, (sp + 1) * SBS):
            ob_t = work.tile([128, DM], F32, name=f"ob_{qb}", tag="ob", bufs=2)
            for nh in range(NOUT):
                pot = psum.tile([128, OW], F32, name=f"pot_{qb}_{nh}", tag="po", bufs=2)
                for c in range(NHD):
                    nc.tensor.matmul(
                        pot[:],
                        OT[c][:, qb * SB : (qb + 1) * SB],
                        wo[:, c * DM + nh * OW : c * DM + (nh + 1) * OW],
                        start=(c == 0),
                        stop=(c == NHD - 1),
                        skip_group_check=True,
                    )
                    yield
                nc.vector.tensor_copy(ob_t[:, nh * OW : (nh + 1) * OW], pot[:])
            nc.sync.dma_start(io["out"][qb * SB : (qb + 1) * SB, :], ob_t[:])
            yield

    # Two-level filler queue: "prio" holds next-span projections (must finish
    # before that span's attention), "bulk" holds out-projections (no deadline
    # until the end — saved to keep the PE fed through span 3's long passes).
    prio = []
    bulk = []

    def pull(n):
        k = 0
        while k < n:
            q = prio if prio else bulk
            if not q:
                return
            try:
                next(q[0])
                k += 1
            except StopIteration:
                q.pop(0)

    def exhaust(g):
        while True:
            try:
                next(g)
            except StopIteration:
                break
        if g in prio:
            prio.remove(g)
        if g in bulk:
            bulk.remove(g)

    def drain_all():
        while prio or bulk:
            pull(64)

    # ---- attention for one span, two 2-head passes ----
    def attention_span(sp):
        nsb = (sp + 1) * SBS
        for p in range(2):
            hA = 2 * p
            pts = {}
            pos = {}

            def emit_scores(i):
                sb = i
                d = sb - sp * SBS
                offc = 0 if d < 1 else (128 * d if d < 3 else 256)
                offe = 0 if d < 1 else 128 * d
                pt = work.tile(
                    [128, 2 * SPAN], BF16, name=f"pt_{sp}_{p}_{i}", tag="pt", bufs=3
                )
                pts[i] = (pt, offe)
                s_t = psum.tile(
                    [128, 2 * SPAN], F32, name=f"ps_{sp}_{p}_{i}", tag="sc", bufs=2
                )
                for j in range(2):
                    h = hA + j
                    qt = qkT[h // 2]
                    kt = qkT[NQK // 2 + h // 2]
                    qrow = (h % 2) * 64
                    nc.tensor.matmul(
                        s_t[:, j * SPAN + offc : (j + 1) * SPAN],
                        kt[qrow : qrow + 64, sb * SB : (sb + 1) * SB],
                        qt[qrow : qrow + 64, sp * SPAN + offc : (sp + 1) * SPAN],
                        start=True,
                        stop=True,
                        skip_group_check=True,
                    )
                # one exp covers both heads' (trimmed) score blocks
                nc.scalar.activation(
                    pt[:].rearrange("p (j w) -> p j w", w=SPAN)[:, :, offe:SPAN],
                    s_t[:].rearrange("p (j w) -> p j w", w=SPAN)[:, :, offe:SPAN],
                    AF.Exp,
                    scale=0.125,
                )
                if d >= 0:
                    ptv = pt[:].rearrange("p (j w) -> p j w", w=SPAN)[
                        :, :, 128 * d : 128 * (d + 1)
                    ]
                    nc.vector.tensor_mul(
                        ptv, ptv, tri2[:].rearrange("p (j w) -> p j w", w=SB)
                    )

            def emit_pv(i):
                sb = i
                # emission-order guard: vp[sb]'s writer must be emitted first
                while not vp_ready[sb]:
                    pull(8)
                d = sb - sp * SBS
                offe = 0 if d < 1 else 128 * d
                pt, _ = pts.pop(i)
                for j in range(2):
                    h = hA + j
                    if i == 0:
                        pos[h] = psum.tile(
                            [VW, SPAN], F32, name=f"po_{sp}_{h}", tag="pa", bufs=2
                        )
                    dst = pos[h][:, offe:SPAN] if offe else pos[h][:]
                    nc.tensor.matmul(
                        dst,
                        vp[sb][:, h * VW : (h + 1) * VW],
                        pt[:, j * SPAN + offe : (j + 1) * SPAN],
                        start=(i == 0),
                        stop=(i == nsb - 1),
                        skip_group_check=True,
                    )

            if p == 1 and sp in p1b:
                exhaust(p1b[sp])  # pass B scores need qkT obs 1,3 of this span
            pull(4)
            # PV lags scores by 2 steps: both PE gates (scores' WAR on exp,
            # PV's RAW on exp) get a >=2-cycle window, so semaphore jitter
            # never stalls the PE sequencer (which would reset the p-state).
            for i in range(nsb + 2):
                if i < nsb:
                    emit_scores(i)
                pull(2)
                if i >= 2:
                    emit_pv(i - 2)

            # normalization: out^T rows 0..63 divided by the denominator row
            for j in range(2):
                h = hA + j
                den = work.tile([1, SPAN], F32, name=f"den_{sp}_{h}", tag="den", bufs=2)
                nc.vector.tensor_copy(den[0:1, :], pos[h][VW - 1 : VW, :])
                rden = work.tile([1, SPAN], F32, name=f"rden_{sp}_{h}", tag="rden", bufs=2)
                nc.vector.reciprocal_approx_fast(rden[0:1, :], den[0:1, :])
                recb = work.tile([DH, SPAN], F32, name=f"recb_{sp}_{h}", tag="recb", bufs=2)
                nc.gpsimd.partition_broadcast(recb[:], rden[0:1, :])
                nc.vector.tensor_mul(
                    OT[h // 2][(h % 2) * DH : (h % 2 + 1) * DH, sp * SPAN : (sp + 1) * SPAN],
                    pos[h][0:DH, :],
                    recb[:],
                )

    # ---- span-0 projections, contraction-chunk OUTER so each x/w chunk is
    # consumed as its DMA lands (the attention "sc" and "po" psum banks are
    # free this early, giving 4 qk + 2x2 v accumulators) ----
    # pqk0: ob pairs packed into 2-bank "sc" tiles (independent accumulation
    # groups may share a TILE but never a BANK — halves sit in separate banks)
    pqk0 = [psum.tile([128, 2 * SPAN], F32, name=f"pqk0_{i}", tag="sc", bufs=2) for i in range(2)]
    pv0 = [
        psum.tile([128, DQ], F32, name=f"pv0_{sb}", tag=("po" if sb < 2 else "pa"), bufs=2)
        for sb in range(SBS)
    ]
    for c in range(NDM):
        for obi in range(NQK):
            nc.tensor.matmul(
                pqk0[obi // 2][:, (obi % 2) * SPAN : (obi % 2 + 1) * SPAN],
                wqk[:, c * 2 * DQ + obi * 128 : c * 2 * DQ + (obi + 1) * 128],
                xT[:, c * S : c * S + SPAN],
                start=(c == 0),
                stop=(c == NDM - 1),
                skip_group_check=True,
            )
        for sb in range(SBS):
            nc.tensor.matmul(
                pv0[sb][:],
                xT[:, c * S + sb * SB : c * S + (sb + 1) * SB],
                wv[:, c * DQ : (c + 1) * DQ],
                start=(c == 0),
                stop=(c == NDM - 1),
                skip_group_check=True,
            )
    for obi in (0, 2, 1, 3):  # pass-A tiles (q heads 0/1, k heads 0/1) first
        nc.vector.tensor_scalar_add(
            qkT[obi][:, 0:SPAN],
            pqk0[obi // 2][:, (obi % 2) * SPAN : (obi % 2 + 1) * SPAN],
            bqk[:, obi : obi + 1],
        )
    for sb in range(SBS):
        vdst = vp[sb][:, :].rearrange("p (h w) -> p h w", w=VW)[:, :, 0:DH]
        nc.vector.tensor_add(
            vdst,
            pv0[sb][:].rearrange("p (h d) -> p h d", d=DH),
            bv[:].rearrange("p (h d) -> p h d", d=DH),
        )
        nc.vector.memset(vp[sb][:, DH : HPC * VW : VW], 1.0)
        vp_ready[sb] = True
    for sp in range(NSPAN):
        if sp + 1 < NSPAN:
            p1a = gen_phase1(sp + 1, (0, 2))
            p1b[sp + 1] = gen_phase1(sp + 1, (1, 3))
            prio.append(p1a)
            prio.append(p1b[sp + 1])
            prio.append(gen_phase2(sp + 1))
        attention_span(sp)
        if sp + 1 < NSPAN:
            exhaust(p1a)  # next span's pass A needs qkT obs 0,2 emitted
        if sp < 2:
            bulk.append(gen_outproj(sp))
    # op(2) held back: its 16 ready-to-run matmuls keep the PE hot across the
    # final normalization chain; op(3) follows right behind.
    bulk.append(gen_outproj(2))
    bulk.append(gen_outproj(3))
    drain_all()


_NC_CACHE = {}


def _get_compiled():
    if "nc" not in _NC_CACHE:
        nc = bacc.Bacc(
            "TRN2", target_bir_lowering=False, debug=False, num_devices=N_CORES
        )
        io = _declare_io(nc)
        with tile.TileContext(nc) as tc, ExitStack() as ctx:
            _build(ctx, tc, io)
        nc.compile()
        _NC_CACHE["nc"] = nc
    return _NC_CACHE["nc"]


def _prep_core_inputs(x, W_qkv, b_qkv, W_out, b_out, core_id):
    g = core_id // CPG
    lane = core_id % CPG
    h0 = lane * HPC
    r = slice(h0 * DH, (h0 + HPC) * DH)
    Wq = W_qkv[0 * DM : 1 * DM, :][r, :]
    Wk = W_qkv[1 * DM : 2 * DM, :][r, :]
    Wv = W_qkv[2 * DM : 3 * DM, :][r, :]
    bq = b_qkv[0 * DM + h0 * DH : 0 * DM + (h0 + HPC) * DH]
    bk = b_qkv[1 * DM + h0 * DH : 1 * DM + (h0 + HPC) * DH]
    bv_ = b_qkv[2 * DM + h0 * DH : 2 * DM + (h0 + HPC) * DH]
    return {
        "xT": np.ascontiguousarray(x[g].T.astype(ml_dtypes.bfloat16)),
        "wqkT": np.ascontiguousarray(
            np.concatenate([Wq.T, Wk.T], axis=1).astype(ml_dtypes.bfloat16)
        ),
        "wvT": np.ascontiguousarray(Wv.T.astype(ml_dtypes.bfloat16)),
        "woT": np.ascontiguousarray(W_out[:, r].T.astype(np.float32)),
        "bqk": np.concatenate([bq, bk]).reshape(2 * DQ, 1).astype(np.float32),
        "bv": np.ascontiguousarray(
            np.broadcast_to(bv_.reshape(1, DQ), (128, DQ)).astype(np.float32)
        ),
    }


def kernel(x, W_qkv, b_qkv, W_out, b_out, _trace=False):
    x = np.asarray(x)
    W_qkv = np.asarray(W_qkv)
    b_qkv = np.asarray(b_qkv)
    W_out = np.asarray(W_out)
    b_out = np.asarray(b_out)

    nc = _get_compiled()
    in_maps = [
        _prep_core_inputs(x, W_qkv, b_qkv, W_out, b_out, c) for c in range(N_CORES)
    ]
    res = run_bass_kernel_spmd(nc, in_maps, list(range(N_CORES)), trace=_trace)

    out = np.empty((B, S, DM), dtype=np.float32)
    for g in range(B):
        acc = res.results[g * CPG]["out"].astype(np.float32)
        for lane in range(1, CPG):
            acc = acc + res.results[g * CPG + lane]["out"]
        out[g] = acc + b_out[None, :].astype(np.float32)

    if _trace:
        kernel.last_exec_time_ns = res.exec_time_ns
        kernel.last_results = res
    return out


# revision 29
# speedup vs baseline: 1.7503x; 1.0471x over previous
"""Multi-head causal self-attention (torch nn.MultiheadAttention semantics)
on 8 Trainium2 NeuronCores.

Problem: x [2, 2048, 1024], 16 heads, head dim 64, fp32, causal, p_drop=0.

Sharding: 2 batch groups x 4-way head tensor-parallel.
  core c: batch b = c // 4, heads [lane*4, lane*4+4) with lane = c % 4.
Each core computes q/k/v projections for its 4 heads, flash-style causal
attention (S^T score layout, no-max softmax), and its partial out-projection.
The host sums the 4 partials per batch and adds b_out.

Performance structure (v2): the whole per-core program is emitted as ONE
software-pipelined instruction stream so the PE sequencer never blocks on a
semaphore. A blocked PE sequencer serializes the next LDWEIGHTS behind the
current matmul AND holds the PE at the 1.2 GHz mid p-state; back-to-back
matmuls hide LDWEIGHTS entirely and ramp the PE to 2.4 GHz after ~3 us
(measured: 230 ns vs 800+ ns per 512-row f32r matmul).

  - Attention runs in 2-head passes (scores psum "sc" 4x[128,512] rotation,
    PV accumulators "pa" 2x[65,512], projections/out-proj "po" 2x[128,512]
    = exactly 8 PSUM banks).
  - q/k/v projections of span sp+1 and out-projection of span sp-1 are
    generators whose matmuls are pulled 2-per-attention-step as PE filler
    between a step's score matmuls and the previous step's PV matmuls,
    covering the exp (Act) latency.
  - exp outputs P^T in bf16 (Act speed is dtype-independent; DVE tri-mask
    gets 2x; PV runs 1 cycle/row at any width, enabling causal trimming of
    the diagonal blocks on both the scores and PV matmuls).
  - softmax denominator rides as a 65th "ones" column of V'; normalization
    is reciprocal_approx_fast + gpsimd partition-broadcast + DVE mul.
"""

import ml_dtypes
import numpy as np
from contextlib import ExitStack

import concourse.bass as bass
import concourse.tile as tile
from concourse import bacc, mybir
from concourse.bass_utils import run_bass_kernel_spmd

F32 = mybir.dt.float32
F32R = mybir.dt.float32r
BF16 = mybir.dt.bfloat16
AF = mybir.ActivationFunctionType

B = 2
S = 2048
DM = 1024
N_HEADS = 16
DH = 64
N_CORES = 8
CPG = 4                    # cores per group (tensor-parallel width)
HPC = N_HEADS // CPG       # 4 heads per core
DQ = HPC * DH              # 256
SPAN = 512
SB = 128
NDM = DM // 128            # 8 x/weight chunks
NSPAN = S // SPAN          # 4
SBS = SPAN // SB           # 4
NSB = S // SB              # 16
NQK = 2 * DQ // 128        # 4 qkT row tiles
NHD = DQ // 128            # 2 OT row tiles
VW = DH + 1                # 65: per-head V plus ones column
OW = 512
NOUT = DM // OW            # 2


def _declare_io(nc):
    t = {}
    t["xT"] = nc.dram_tensor("xT", [DM, S], BF16, kind="ExternalInput").ap()
    t["wqkT"] = nc.dram_tensor("wqkT", [DM, 2 * DQ], BF16, kind="ExternalInput").ap()
    t["wvT"] = nc.dram_tensor("wvT", [DM, DQ], BF16, kind="ExternalInput").ap()
    t["woT"] = nc.dram_tensor("woT", [DQ, DM], BF16, kind="ExternalInput").ap()
    t["bqk"] = nc.dram_tensor("bqk", [2 * DQ, 1], F32, kind="ExternalInput").ap()
    t["bv"] = nc.dram_tensor("bv", [128, DQ], F32, kind="ExternalInput").ap()
    t["out"] = nc.dram_tensor("out", [S, DM], F32, kind="ExternalOutput").ap()
    return t


def _build(ctx: ExitStack, tc: tile.TileContext, io: dict):
    nc = tc.nc

    const = ctx.enter_context(tc.tile_pool(name="const", bufs=1))
    work = ctx.enter_context(tc.tile_pool(name="work", bufs=1))
    psum = ctx.enter_context(tc.tile_pool(name="psum", bufs=1, space="PSUM"))

    # ---- input DMAs: ordered so phase1(span 0) can start ~immediately ----
    wv = const.tile([128, NDM * DQ], BF16, name="wv")
    nc.scalar.dma_start(
        wv[:].rearrange("p (c w) -> p c w", w=DQ),
        io["wvT"].rearrange("(c p) w -> p c w", p=128),
    )
    bv = const.tile([128, DQ], F32, name="bv")
    nc.scalar.dma_start(bv[:], io["bv"][:])
    bqk = const.tile([128, NQK], F32, name="bqk")
    for obi in range(NQK):
        nc.scalar.dma_start(bqk[:, obi : obi + 1], io["bqk"][obi * 128 : (obi + 1) * 128, :])

    wqk = const.tile([128, NDM * 2 * DQ], BF16, name="wqk")
    xT = const.tile([128, NDM * S], BF16, name="xT")
    # half-batched loads: chunks 0-3 then 4-7, x-span0 ahead of wqk
    for lo in (0, 4):
        nc.sync.dma_start(
            xT[:].rearrange("p (c s) -> p c s", s=S)[:, lo : lo + 4, 0:SPAN],
            io["xT"].rearrange("(c p) s -> p c s", p=128)[:, lo : lo + 4, 0:SPAN],
        )
        nc.sync.dma_start(
            wqk[:].rearrange("p (c w) -> p c w", w=2 * DQ)[:, lo : lo + 4, :],
            io["wqkT"].rearrange("(c p) w -> p c w", p=128)[:, lo : lo + 4, :],
        )
    for sp in range(1, NSPAN):
        nc.sync.dma_start(
            xT[:].rearrange("p (c s) -> p c s", s=S)[:, :, sp * SPAN : (sp + 1) * SPAN],
            io["xT"].rearrange("(c p) s -> p c s", p=128)[:, :, sp * SPAN : (sp + 1) * SPAN],
        )
    wo = const.tile([128, NHD * DM], BF16, name="wo")
    nc.scalar.dma_start(
        wo[:].rearrange("p (c w) -> p c w", w=DM),
        io["woT"].rearrange("(c p) w -> p c w", p=128),
    )

    # causal triangle for one diagonal 128x128 sub-block, duplicated for the
    # 2-head strided multiply: tri2[r, j*128+c] = (c - r >= 0)
    tri2 = const.tile([128, 2 * SB], BF16, name="tri2")
    nc.gpsimd.memset(tri2[:], 1.0)
    for half in range(2):
        nc.gpsimd.affine_select(
            out=tri2[:, half * SB : (half + 1) * SB],
            in_=tri2[:, half * SB : (half + 1) * SB],
            compare_op=mybir.AluOpType.is_ge,
            fill=0.0,
            base=0,
            pattern=[[1, SB]],
            channel_multiplier=-1,
        )

    # ---- persistent activations ----
    qkT = [const.tile([128, S], BF16, name=f"qkT{b}") for b in range(NQK)]
    vp = [const.tile([128, HPC * VW], BF16, name=f"vp{sb}") for sb in range(NSB)]
    OT = [const.tile([128, S], BF16, name=f"OT{c}") for c in range(NHD)]

    vp_ready = [False] * NSB
    p1b = {}  # span -> generator for that span's pass-B qk projections

    # ---- filler generators (each yield = one PE matmul emitted) ----
    def gen_phase1(sp, obs):
        for obi in obs:
            pqk = psum.tile([128, SPAN], F32, name=f"pqk_{sp}_{obi}", tag="po", bufs=2)
            for c in range(NDM):
                nc.tensor.matmul(
                    pqk[:],
                    wqk[:, c * 2 * DQ + obi * 128 : c * 2 * DQ + (obi + 1) * 128],
                    xT[:, c * S + sp * SPAN : c * S + (sp + 1) * SPAN],
                    start=(c == 0),
                    stop=(c == NDM - 1),
                    skip_group_check=True,
                )
                if c < NDM - 1:
                    yield
            nc.vector.tensor_scalar_add(
                qkT[obi][:, sp * SPAN : (sp + 1) * SPAN], pqk[:], bqk[:, obi : obi + 1]
            )
            yield

    def gen_phase2(sp):
        for sb in range(sp * SBS, (sp + 1) * SBS):
            pv = psum.tile([128, DQ], F32, name=f"pv_{sb}", tag="po", bufs=2)
            for c in range(NDM):
                nc.tensor.matmul(
                    pv[:],
                    xT[:, c * S + sb * SB : c * S + (sb + 1) * SB],
                    wv[:, c * DQ : (c + 1) * DQ],
                    start=(c == 0),
                    stop=(c == NDM - 1),
                    skip_group_check=True,
                )
                if c < NDM - 1:
                    yield
            vdst = vp[sb][:, :].rearrange("p (h w) -> p h w", w=VW)[:, :, 0:DH]
            nc.vector.tensor_add(
                vdst,
                pv[:].rearrange("p (h d) -> p h d", d=DH),
                bv[:].rearrange("p (h d) -> p h d", d=DH),
            )
            ones = vp[sb][:, DH : HPC * VW : VW]
            nc.vector.memset(ones, 1.0)
            vp_ready[sb] = True
            yield

    def gen_outproj(sp):
        for qb in range(sp * SBS, (sp + 1) * SBS):
            ob_t = work.tile([128, DM], F32, name=f"ob_{qb}", tag="ob", bufs=2)
            for nh in range(NOUT):
                pot = psum.tile([128, OW], F32, name=f"pot_{qb}_{nh}", tag="po", bufs=2)
                for c in range(NHD):
                    nc.tensor.matmul(
                        pot[:],
                        OT[c][:, qb * SB : (qb + 1) * SB],
                        wo[:, c * DM + nh * OW : c * DM + (nh + 1) * OW],
                        start=(c == 0),
                        stop=(c == NHD - 1),
                        skip_group_check=True,
                    )
                    yield
                nc.vector.tensor_copy(ob_t[:, nh * OW : (nh + 1) * OW], pot[:])
            nc.sync.dma_start(io["out"][qb * SB : (qb + 1) * SB, :], ob_t[:])
            yield

    # Two-level filler queue: "prio" holds next-span projections (must finish
    # before that span's attention), "bulk" holds out-projections (no deadline
    # until the end — saved to keep the PE fed through span 3's long passes).
    prio = []
    bulk = []

    def pull(n):
        k = 0
        while k < n:
            q = prio if prio else bulk
            if not q:
                return
            try:
                next(q[0])
                k += 1
            except StopIteration:
                q.pop(0)

    def exhaust(g):
        while True:
            try:
                next(g)
            except StopIteration:
                break
        if g in prio:
            prio.remove(g)
        if g in bulk:
            bulk.remove(g)

    def drain_all():
        while prio or bulk:
            pull(64)

    # ---- attention for one span, two 2-head passes ----
    def attention_span(sp):
        nsb = (sp + 1) * SBS
        for p in range(2):
            hA = 2 * p
            pts = {}
            pos = {}

            def emit_scores(i):
                sb = i
                d = sb - sp * SBS
                offc = offe = 0 if d < 1 else 128 * d
                pt = work.tile(
                    [128, 2 * SPAN], BF16, name=f"pt_{sp}_{p}_{i}", tag="pt", bufs=3
                )
                pts[i] = (pt, offe)
                s_t = psum.tile(
                    [128, 2 * SPAN], F32, name=f"ps_{sp}_{p}_{i}", tag="sc", bufs=2
                )
                for j in range(2):
                    h = hA + j
                    qt = qkT[h // 2]
                    kt = qkT[NQK // 2 + h // 2]
                    qrow = (h % 2) * 64
                    nc.tensor.matmul(
                        s_t[:, j * SPAN + offe : (j + 1) * SPAN],
                        kt[qrow : qrow + 64, sb * SB : (sb + 1) * SB],
                        qt[qrow : qrow + 64, sp * SPAN + offe : (sp + 1) * SPAN],
                        start=True,
                        stop=True,
                        skip_group_check=True,
                    )
                # one exp covers both heads' (trimmed) score blocks
                nc.scalar.activation(
                    pt[:].rearrange("p (j w) -> p j w", w=SPAN)[:, :, offe:SPAN],
                    s_t[:].rearrange("p (j w) -> p j w", w=SPAN)[:, :, offe:SPAN],
                    AF.Exp,
                    scale=0.125,
                )
                if d >= 0:
                    ptv = pt[:].rearrange("p (j w) -> p j w", w=SPAN)[
                        :, :, 128 * d : 128 * (d + 1)
                    ]
                    nc.vector.tensor_mul(
                        ptv, ptv, tri2[:].rearrange("p (j w) -> p j w", w=SB)
                    )

            def emit_pv(i):
                sb = i
                # emission-order guard: vp[sb]'s writer must be emitted first
                while not vp_ready[sb]:
                    pull(8)
                d = sb - sp * SBS
                offe = 0 if d < 1 else 128 * d
                pt, _ = pts.pop(i)
                for j in range(2):
                    h = hA + j
                    if i == 0:
                        pos[h] = psum.tile(
                            [VW, SPAN], F32, name=f"po_{sp}_{h}", tag="pa", bufs=2
                        )
                    dst = pos[h][:, offe:SPAN] if offe else pos[h][:]
                    nc.tensor.matmul(
                        dst,
                        vp[sb][:, h * VW : (h + 1) * VW],
                        pt[:, j * SPAN + offe : (j + 1) * SPAN],
                        start=(i == 0),
                        stop=(i == nsb - 1),
                        skip_group_check=True,
                    )

            if p == 1 and sp in p1b:
                exhaust(p1b[sp])  # pass B scores need qkT obs 1,3 of this span
            pull(4)
            # PV lags scores by 2 steps: both PE gates (scores' WAR on exp,
            # PV's RAW on exp) get a >=2-cycle window, so semaphore jitter
            # never stalls the PE sequencer (which would reset the p-state).
            for i in range(nsb + 2):
                if i < nsb:
                    emit_scores(i)
                pull(4 if i % 2 == 0 else 0)
                if i >= 2:
                    emit_pv(i - 2)

            # normalization: out^T rows 0..63 divided by the denominator row
            for j in range(2):
                h = hA + j
                den = work.tile([1, SPAN], F32, name=f"den_{sp}_{h}", tag="den", bufs=2)
                nc.vector.tensor_copy(den[0:1, :], pos[h][VW - 1 : VW, :])
                rden = work.tile([1, SPAN], F32, name=f"rden_{sp}_{h}", tag="rden", bufs=2)
                nc.vector.reciprocal_approx_fast(rden[0:1, :], den[0:1, :])
                recb = work.tile([DH, SPAN], F32, name=f"recb_{sp}_{h}", tag="recb", bufs=2)
                nc.gpsimd.partition_broadcast(recb[:], rden[0:1, :])
                nc.vector.tensor_mul(
                    OT[h // 2][(h % 2) * DH : (h % 2 + 1) * DH, sp * SPAN : (sp + 1) * SPAN],
                    pos[h][0:DH, :],
                    recb[:],
                )

    # ---- span-0 projections, contraction-chunk OUTER so each x/w chunk is
    # consumed as its DMA lands (the attention "sc" and "po" psum banks are
    # free this early, giving 4 qk + 2x2 v accumulators) ----
    # pqk0: ob pairs packed into 2-bank "sc" tiles (independent accumulation
    # groups may share a TILE but never a BANK — halves sit in separate banks)
    pqk0 = [psum.tile([128, 2 * SPAN], F32, name=f"pqk0_{i}", tag="sc", bufs=2) for i in range(2)]
    pv0 = [
        psum.tile([128, DQ], F32, name=f"pv0_{sb}", tag=("po" if sb < 2 else "pa"), bufs=2)
        for sb in range(SBS)
    ]
    for c in range(NDM):
        for obi in range(NQK):
            nc.tensor.matmul(
                pqk0[obi // 2][:, (obi % 2) * SPAN : (obi % 2 + 1) * SPAN],
                wqk[:, c * 2 * DQ + obi * 128 : c * 2 * DQ + (obi + 1) * 128],
                xT[:, c * S : c * S + SPAN],
                start=(c == 0),
                stop=(c == NDM - 1),
                skip_group_check=True,
            )
        for sb in range(SBS):
            nc.tensor.matmul(
                pv0[sb][:],
                xT[:, c * S + sb * SB : c * S + (sb + 1) * SB],
                wv[:, c * DQ : (c + 1) * DQ],
                start=(c == 0),
                stop=(c == NDM - 1),
                skip_group_check=True,
            )
    for obi in (0, 2, 1, 3):  # pass-A tiles (q heads 0/1, k heads 0/1) first
        nc.vector.tensor_scalar_add(
            qkT[obi][:, 0:SPAN],
            pqk0[obi // 2][:, (obi % 2) * SPAN : (obi % 2 + 1) * SPAN],
            bqk[:, obi : obi + 1],
        )
    for sb in range(SBS):
        vdst = vp[sb][:, :].rearrange("p (h w) -> p h w", w=VW)[:, :, 0:DH]
        nc.vector.tensor_add(
            vdst,
            pv0[sb][:].rearrange("p (h d) -> p h d", d=DH),
            bv[:].rearrange("p (h d) -> p h d", d=DH),
        )
        nc.vector.memset(vp[sb][:, DH : HPC * VW : VW], 1.0)
        vp_ready[sb] = True
    for sp in range(NSPAN):
        if sp + 1 < NSPAN:
            p1a = gen_phase1(sp + 1, (0, 2))
            p1b[sp + 1] = gen_phase1(sp + 1, (1, 3))
            prio.append(p1a)
            prio.append(p1b[sp + 1])
            prio.append(gen_phase2(sp + 1))
        attention_span(sp)
        if sp + 1 < NSPAN:
            exhaust(p1a)  # next span's pass A needs qkT obs 0,2 emitted
        if sp < 2:
            bulk.append(gen_outproj(sp))
    # op(2) held back: its 16 ready-to-run matmuls keep the PE hot across the
    # final normalization chain; op(3) follows right behind.
    bulk.append(gen_outproj(2))
    bulk.append(gen_outproj(3))
    drain_all()


_NC_CACHE = {}


def _get_compiled():
    if "nc" not in _NC_CACHE:
        nc = bacc.Bacc(
            "TRN2", target_bir_lowering=False, debug=False, num_devices=N_CORES
        )
        io = _declare_io(nc)
        with tile.TileContext(nc) as tc, ExitStack() as ctx:
            _build(ctx, tc, io)
        nc.compile()
        _NC_CACHE["nc"] = nc
    return _NC_CACHE["nc"]


def _prep_core_inputs(x, W_qkv, b_qkv, W_out, b_out, core_id):
    g = core_id // CPG
    lane = core_id % CPG
    h0 = lane * HPC
    r = slice(h0 * DH, (h0 + HPC) * DH)
    Wq = W_qkv[0 * DM : 1 * DM, :][r, :]
    Wk = W_qkv[1 * DM : 2 * DM, :][r, :]
    Wv = W_qkv[2 * DM : 3 * DM, :][r, :]
    bq = b_qkv[0 * DM + h0 * DH : 0 * DM + (h0 + HPC) * DH]
    bk = b_qkv[1 * DM + h0 * DH : 1 * DM + (h0 + HPC) * DH]
    bv_ = b_qkv[2 * DM + h0 * DH : 2 * DM + (h0 + HPC) * DH]
    return {
        "xT": np.ascontiguousarray(x[g].T.astype(ml_dtypes.bfloat16)),
        "wqkT": np.ascontiguousarray(
            np.concatenate([Wq.T, Wk.T], axis=1).astype(ml_dtypes.bfloat16)
        ),
        "wvT": np.ascontiguousarray(Wv.T.astype(ml_dtypes.bfloat16)),
        "woT": np.ascontiguousarray(W_out[:, r].T.astype(ml_dtypes.bfloat16)),
        "bqk": np.concatenate([bq, bk]).reshape(2 * DQ, 1).astype(np.float32),
        "bv": np.ascontiguousarray(
            np.broadcast_to(bv_.reshape(1, DQ), (128, DQ)).astype(np.float32)
        ),
    }


def kernel(x, W_qkv, b_qkv, W_out, b_out, _trace=False):
    x = np.asarray(x)
    W_qkv = np.asarray(W_qkv)
    b_qkv = np.asarray(b_qkv)
    W_out = np.asarray(W_out)
    b_out = np.asarray(b_out)

    nc = _get_compiled()
    in_maps = [
        _prep_core_inputs(x, W_qkv, b_qkv, W_out, b_out, c) for c in range(N_CORES)
    ]
    res = run_bass_kernel_spmd(nc, in_maps, list(range(N_CORES)), trace=_trace)

    out = np.empty((B, S, DM), dtype=np.float32)
    for g in range(B):
        acc = res.results[g * CPG]["out"].astype(np.float32)
        for lane in range(1, CPG):
            acc = acc + res.results[g * CPG + lane]["out"]
        out[g] = acc + b_out[None, :].astype(np.float32)

    if _trace:
        kernel.last_exec_time_ns = res.exec_time_ns
        kernel.last_results = res
    return out


# revision 30
# speedup vs baseline: 1.7963x; 1.0263x over previous
"""Multi-head causal self-attention (torch nn.MultiheadAttention semantics)
on 8 Trainium2 NeuronCores.

Problem: x [2, 2048, 1024], 16 heads, head dim 64, fp32, causal, p_drop=0.

Sharding: 2 batch groups x 4-way head tensor-parallel.
  core c: batch b = c // 4, heads [lane*4, lane*4+4) with lane = c % 4.
Each core computes q/k/v projections for its 4 heads, flash-style causal
attention (S^T score layout, no-max softmax), and its partial out-projection.
The host sums the 4 partials per batch and adds b_out.

Performance structure (v2): the whole per-core program is emitted as ONE
software-pipelined instruction stream so the PE sequencer never blocks on a
semaphore. A blocked PE sequencer serializes the next LDWEIGHTS behind the
current matmul AND holds the PE at the 1.2 GHz mid p-state; back-to-back
matmuls hide LDWEIGHTS entirely and ramp the PE to 2.4 GHz after ~3 us
(measured: 230 ns vs 800+ ns per 512-row f32r matmul).

  - Attention runs in 2-head passes (scores psum "sc" 4x[128,512] rotation,
    PV accumulators "pa" 2x[65,512], projections/out-proj "po" 2x[128,512]
    = exactly 8 PSUM banks).
  - q/k/v projections of span sp+1 and out-projection of span sp-1 are
    generators whose matmuls are pulled 2-per-attention-step as PE filler
    between a step's score matmuls and the previous step's PV matmuls,
    covering the exp (Act) latency.
  - exp outputs P^T in bf16 (Act speed is dtype-independent; DVE tri-mask
    gets 2x; PV runs 1 cycle/row at any width, enabling causal trimming of
    the diagonal blocks on both the scores and PV matmuls).
  - softmax denominator rides as a 65th "ones" column of V'; normalization
    is reciprocal_approx_fast + gpsimd partition-broadcast + DVE mul.
"""

import ml_dtypes
import numpy as np
from contextlib import ExitStack

import concourse.bass as bass
import concourse.tile as tile
from concourse import bacc, mybir
from concourse.bass_utils import run_bass_kernel_spmd

F32 = mybir.dt.float32
F32R = mybir.dt.float32r
BF16 = mybir.dt.bfloat16
AF = mybir.ActivationFunctionType

B = 2
S = 2048
DM = 1024
N_HEADS = 16
DH = 64
N_CORES = 8
CPG = 4                    # cores per group (tensor-parallel width)
HPC = N_HEADS // CPG       # 4 heads per core
DQ = HPC * DH              # 256
SPAN = 512
SB = 128
NDM = DM // 128            # 8 x/weight chunks
NSPAN = S // SPAN          # 4
SBS = SPAN // SB           # 4
NSB = S // SB              # 16
NQK = 2 * DQ // 128        # 4 qkT row tiles
NHD = DQ // 128            # 2 OT row tiles
VW = DH + 1                # 65: per-head V plus ones column
OW = 512
NOUT = DM // OW            # 2


def _declare_io(nc):
    t = {}
    t["xT"] = nc.dram_tensor("xT", [DM, S], BF16, kind="ExternalInput").ap()
    t["wqkT"] = nc.dram_tensor("wqkT", [DM, 2 * DQ], BF16, kind="ExternalInput").ap()
    t["wvT"] = nc.dram_tensor("wvT", [DM, DQ], BF16, kind="ExternalInput").ap()
    t["woT"] = nc.dram_tensor("woT", [DQ, DM], BF16, kind="ExternalInput").ap()
    t["bqk"] = nc.dram_tensor("bqk", [2 * DQ, 1], F32, kind="ExternalInput").ap()
    t["bv"] = nc.dram_tensor("bv", [128, DQ], F32, kind="ExternalInput").ap()
    t["out"] = nc.dram_tensor("out", [S, DM], BF16, kind="ExternalOutput").ap()
    return t


def _build(ctx: ExitStack, tc: tile.TileContext, io: dict):
    nc = tc.nc

    const = ctx.enter_context(tc.tile_pool(name="const", bufs=1))
    work = ctx.enter_context(tc.tile_pool(name="work", bufs=1))
    psum = ctx.enter_context(tc.tile_pool(name="psum", bufs=1, space="PSUM"))

    # ---- PE warm-up: dependency-free matmuls on a memset tile keep the PE
    # busy through the initial input-DMA wait, so the p-state ramp (0.65 ->
    # 1.2 -> 2.4 GHz after 3us continuous) completes before real work ----
    dummy = const.tile([128, SPAN], BF16, name="dummy")
    nc.gpsimd.memset(dummy[:], 0.0)
    pwarm = psum.tile([128, SPAN], F32, name="pwarm", tag="sc", bufs=2)
    for _ in range(24):
        nc.tensor.matmul(pwarm[:], dummy[:, 0:128], dummy[:], start=True, stop=True,
                         skip_group_check=True)

    # ---- input DMAs: ordered so phase1(span 0) can start ~immediately ----
    wv = const.tile([128, NDM * DQ], BF16, name="wv")
    nc.scalar.dma_start(
        wv[:].rearrange("p (c w) -> p c w", w=DQ),
        io["wvT"].rearrange("(c p) w -> p c w", p=128),
    )
    bv = const.tile([128, DQ], F32, name="bv")
    nc.scalar.dma_start(bv[:], io["bv"][:])
    bqk = const.tile([128, NQK], F32, name="bqk")
    for obi in range(NQK):
        nc.scalar.dma_start(bqk[:, obi : obi + 1], io["bqk"][obi * 128 : (obi + 1) * 128, :])

    wqk = const.tile([128, NDM * 2 * DQ], BF16, name="wqk")
    xT = const.tile([128, NDM * S], BF16, name="xT")
    # half-batched loads: chunks 0-3 then 4-7, x-span0 ahead of wqk
    for lo in (0, 4):
        nc.sync.dma_start(
            xT[:].rearrange("p (c s) -> p c s", s=S)[:, lo : lo + 4, 0:SPAN],
            io["xT"].rearrange("(c p) s -> p c s", p=128)[:, lo : lo + 4, 0:SPAN],
        )
        nc.sync.dma_start(
            wqk[:].rearrange("p (c w) -> p c w", w=2 * DQ)[:, lo : lo + 4, :],
            io["wqkT"].rearrange("(c p) w -> p c w", p=128)[:, lo : lo + 4, :],
        )
    for sp in range(1, NSPAN):
        nc.sync.dma_start(
            xT[:].rearrange("p (c s) -> p c s", s=S)[:, :, sp * SPAN : (sp + 1) * SPAN],
            io["xT"].rearrange("(c p) s -> p c s", p=128)[:, :, sp * SPAN : (sp + 1) * SPAN],
        )
    wo = const.tile([128, NHD * DM], BF16, name="wo")
    nc.scalar.dma_start(
        wo[:].rearrange("p (c w) -> p c w", w=DM),
        io["woT"].rearrange("(c p) w -> p c w", p=128),
    )

    # causal triangle for one diagonal 128x128 sub-block, duplicated for the
    # 2-head strided multiply: tri2[r, j*128+c] = (c - r >= 0)
    tri2 = const.tile([128, 2 * SB], BF16, name="tri2")
    nc.gpsimd.memset(tri2[:], 1.0)
    for half in range(2):
        nc.gpsimd.affine_select(
            out=tri2[:, half * SB : (half + 1) * SB],
            in_=tri2[:, half * SB : (half + 1) * SB],
            compare_op=mybir.AluOpType.is_ge,
            fill=0.0,
            base=0,
            pattern=[[1, SB]],
            channel_multiplier=-1,
        )

    # ---- persistent activations ----
    qkT = [const.tile([128, S], BF16, name=f"qkT{b}") for b in range(NQK)]
    vp = [const.tile([128, HPC * VW], BF16, name=f"vp{sb}") for sb in range(NSB)]
    OT = [const.tile([128, S], BF16, name=f"OT{c}") for c in range(NHD)]

    vp_ready = [False] * NSB
    p1b = {}  # span -> generator for that span's pass-B qk projections

    # ---- filler generators (each yield = one PE matmul emitted) ----
    def gen_phase1(sp, obs):
        for obi in obs:
            pqk = psum.tile([128, SPAN], F32, name=f"pqk_{sp}_{obi}", tag="po", bufs=2)
            for c in range(NDM):
                nc.tensor.matmul(
                    pqk[:],
                    wqk[:, c * 2 * DQ + obi * 128 : c * 2 * DQ + (obi + 1) * 128],
                    xT[:, c * S + sp * SPAN : c * S + (sp + 1) * SPAN],
                    start=(c == 0),
                    stop=(c == NDM - 1),
                    skip_group_check=True,
                )
                if c < NDM - 1:
                    yield
            nc.vector.tensor_scalar_add(
                qkT[obi][:, sp * SPAN : (sp + 1) * SPAN], pqk[:], bqk[:, obi : obi + 1]
            )
            yield

    def gen_phase2(sp):
        for sb in range(sp * SBS, (sp + 1) * SBS):
            pv = psum.tile([128, DQ], F32, name=f"pv_{sb}", tag="po", bufs=2)
            for c in range(NDM):
                nc.tensor.matmul(
                    pv[:],
                    xT[:, c * S + sb * SB : c * S + (sb + 1) * SB],
                    wv[:, c * DQ : (c + 1) * DQ],
                    start=(c == 0),
                    stop=(c == NDM - 1),
                    skip_group_check=True,
                )
                if c < NDM - 1:
                    yield
            vdst = vp[sb][:, :].rearrange("p (h w) -> p h w", w=VW)[:, :, 0:DH]
            nc.vector.tensor_add(
                vdst,
                pv[:].rearrange("p (h d) -> p h d", d=DH),
                bv[:].rearrange("p (h d) -> p h d", d=DH),
            )
            ones = vp[sb][:, DH : HPC * VW : VW]
            nc.vector.memset(ones, 1.0)
            vp_ready[sb] = True
            yield

    def gen_outproj(sp):
        for qb in range(sp * SBS, (sp + 1) * SBS):
            ob_t = work.tile([128, DM], BF16, name=f"ob_{qb}", tag="ob", bufs=2)
            for nh in range(NOUT):
                pot = psum.tile([128, OW], F32, name=f"pot_{qb}_{nh}", tag="po", bufs=2)
                for c in range(NHD):
                    nc.tensor.matmul(
                        pot[:],
                        OT[c][:, qb * SB : (qb + 1) * SB],
                        wo[:, c * DM + nh * OW : c * DM + (nh + 1) * OW],
                        start=(c == 0),
                        stop=(c == NHD - 1),
                        skip_group_check=True,
                    )
                    yield
                nc.vector.tensor_copy(ob_t[:, nh * OW : (nh + 1) * OW], pot[:])
            nc.sync.dma_start(io["out"][qb * SB : (qb + 1) * SB, :], ob_t[:])
            yield

    # Two-level filler queue: "prio" holds next-span projections (must finish
    # before that span's attention), "bulk" holds out-projections (no deadline
    # until the end — saved to keep the PE fed through span 3's long passes).
    prio = []
    bulk = []

    def pull(n):
        k = 0
        while k < n:
            q = prio if prio else bulk
            if not q:
                return
            try:
                next(q[0])
                k += 1
            except StopIteration:
                q.pop(0)

    def exhaust(g):
        while True:
            try:
                next(g)
            except StopIteration:
                break
        if g in prio:
            prio.remove(g)
        if g in bulk:
            bulk.remove(g)

    def drain_all():
        while prio or bulk:
            pull(64)

    # ---- attention for one span, two 2-head passes ----
    def attention_span(sp):
        nsb = (sp + 1) * SBS
        for p in range(2):
            hA = 2 * p
            pts = {}
            pos = {}

            def emit_scores(i):
                sb = i
                d = sb - sp * SBS
                offc = offe = 0 if d < 1 else 128 * d
                pt = work.tile(
                    [128, 2 * SPAN], BF16, name=f"pt_{sp}_{p}_{i}", tag="pt", bufs=3
                )
                pts[i] = (pt, offe)
                s_t = psum.tile(
                    [128, 2 * SPAN], F32, name=f"ps_{sp}_{p}_{i}", tag="sc", bufs=2
                )
                for j in range(2):
                    h = hA + j
                    qt = qkT[h // 2]
                    kt = qkT[NQK // 2 + h // 2]
                    qrow = (h % 2) * 64
                    nc.tensor.matmul(
                        s_t[:, j * SPAN + offe : (j + 1) * SPAN],
                        kt[qrow : qrow + 64, sb * SB : (sb + 1) * SB],
                        qt[qrow : qrow + 64, sp * SPAN + offe : (sp + 1) * SPAN],
                        start=True,
                        stop=True,
                        skip_group_check=True,
                    )
                # one exp covers both heads' (trimmed) score blocks
                nc.scalar.activation(
                    pt[:].rearrange("p (j w) -> p j w", w=SPAN)[:, :, offe:SPAN],
                    s_t[:].rearrange("p (j w) -> p j w", w=SPAN)[:, :, offe:SPAN],
                    AF.Exp,
                    scale=0.125,
                )
                if d >= 0:
                    ptv = pt[:].rearrange("p (j w) -> p j w", w=SPAN)[
                        :, :, 128 * d : 128 * (d + 1)
                    ]
                    nc.vector.tensor_mul(
                        ptv, ptv, tri2[:].rearrange("p (j w) -> p j w", w=SB)
                    )

            def emit_pv(i):
                sb = i
                # emission-order guard: vp[sb]'s writer must be emitted first
                while not vp_ready[sb]:
                    pull(8)
                d = sb - sp * SBS
                offe = 0 if d < 1 else 128 * d
                pt, _ = pts.pop(i)
                for j in range(2):
                    h = hA + j
                    if i == 0:
                        pos[h] = psum.tile(
                            [VW, SPAN], F32, name=f"po_{sp}_{h}", tag="pa", bufs=2
                        )
                    dst = pos[h][:, offe:SPAN] if offe else pos[h][:]
                    nc.tensor.matmul(
                        dst,
                        vp[sb][:, h * VW : (h + 1) * VW],
                        pt[:, j * SPAN + offe : (j + 1) * SPAN],
                        start=(i == 0),
                        stop=(i == nsb - 1),
                        skip_group_check=True,
                    )

            if p == 1 and sp in p1b:
                exhaust(p1b[sp])  # pass B scores need qkT obs 1,3 of this span
            pull(4)
            # PV lags scores by 2 steps: both PE gates (scores' WAR on exp,
            # PV's RAW on exp) get a >=2-cycle window, so semaphore jitter
            # never stalls the PE sequencer (which would reset the p-state).
            for i in range(nsb + 2):
                if i < nsb:
                    emit_scores(i)
                pull(8 if i % 4 == 0 else 0)
                if i >= 2:
                    emit_pv(i - 2)

            # normalization: out^T rows 0..63 divided by the denominator row
            for j in range(2):
                h = hA + j
                den = work.tile([1, SPAN], F32, name=f"den_{sp}_{h}", tag="den", bufs=2)
                nc.vector.tensor_copy(den[0:1, :], pos[h][VW - 1 : VW, :])
                rden = work.tile([1, SPAN], F32, name=f"rden_{sp}_{h}", tag="rden", bufs=2)
                nc.vector.reciprocal_approx_fast(rden[0:1, :], den[0:1, :])
                recb = work.tile([DH, SPAN], F32, name=f"recb_{sp}_{h}", tag="recb", bufs=2)
                nc.gpsimd.partition_broadcast(recb[:], rden[0:1, :])
                nc.vector.tensor_mul(
                    OT[h // 2][(h % 2) * DH : (h % 2 + 1) * DH, sp * SPAN : (sp + 1) * SPAN],
                    pos[h][0:DH, :],
                    recb[:],
                )

    # ---- span-0 projections, contraction-chunk OUTER so each x/w chunk is
    # consumed as its DMA lands (the attention "sc" and "po" psum banks are
    # free this early, giving 4 qk + 2x2 v accumulators) ----
    # pqk0: ob pairs packed into 2-bank "sc" tiles (independent accumulation
    # groups may share a TILE but never a BANK — halves sit in separate banks)
    pqk0 = [psum.tile([128, 2 * SPAN], F32, name=f"pqk0_{i}", tag="sc", bufs=2) for i in range(2)]
    pv0 = [
        psum.tile([128, DQ], F32, name=f"pv0_{sb}", tag=("po" if sb < 2 else "pa"), bufs=2)
        for sb in range(SBS)
    ]
    for c in range(NDM):
        for obi in range(NQK):
            nc.tensor.matmul(
                pqk0[obi // 2][:, (obi % 2) * SPAN : (obi % 2 + 1) * SPAN],
                wqk[:, c * 2 * DQ + obi * 128 : c * 2 * DQ + (obi + 1) * 128],
                xT[:, c * S : c * S + SPAN],
                start=(c == 0),
                stop=(c == NDM - 1),
                skip_group_check=True,
            )
        for sb in range(SBS):
            nc.tensor.matmul(
                pv0[sb][:],
                xT[:, c * S + sb * SB : c * S + (sb + 1) * SB],
                wv[:, c * DQ : (c + 1) * DQ],
                start=(c == 0),
                stop=(c == NDM - 1),
                skip_group_check=True,
            )
    for obi in (0, 2, 1, 3):  # pass-A tiles (q heads 0/1, k heads 0/1) first
        nc.vector.tensor_scalar_add(
            qkT[obi][:, 0:SPAN],
            pqk0[obi // 2][:, (obi % 2) * SPAN : (obi % 2 + 1) * SPAN],
            bqk[:, obi : obi + 1],
        )
    for sb in range(SBS):
        vdst = vp[sb][:, :].rearrange("p (h w) -> p h w", w=VW)[:, :, 0:DH]
        nc.vector.tensor_add(
            vdst,
            pv0[sb][:].rearrange("p (h d) -> p h d", d=DH),
            bv[:].rearrange("p (h d) -> p h d", d=DH),
        )
        nc.vector.memset(vp[sb][:, DH : HPC * VW : VW], 1.0)
        vp_ready[sb] = True
    for sp in range(NSPAN):
        if sp + 1 < NSPAN:
            p1a = gen_phase1(sp + 1, (0, 2))
            p1b[sp + 1] = gen_phase1(sp + 1, (1, 3))
            prio.append(p1a)
            prio.append(p1b[sp + 1])
            prio.append(gen_phase2(sp + 1))
        attention_span(sp)
        if sp + 1 < NSPAN:
            exhaust(p1a)  # next span's pass A needs qkT obs 0,2 emitted
        if sp < 2:
            bulk.append(gen_outproj(sp))
    # op(2) held back: its 16 ready-to-run matmuls keep the PE hot across the
    # final normalization chain; op(3) follows right behind.
    bulk.append(gen_outproj(2))
    bulk.append(gen_outproj(3))
    drain_all()


_NC_CACHE = {}


def _get_compiled():
    if "nc" not in _NC_CACHE:
        nc = bacc.Bacc(
            "TRN2", target_bir_lowering=False, debug=False, num_devices=N_CORES
        )
        io = _declare_io(nc)
        with tile.TileContext(nc) as tc, ExitStack() as ctx:
            _build(ctx, tc, io)
        nc.compile()
        _NC_CACHE["nc"] = nc
    return _NC_CACHE["nc"]


def _prep_core_inputs(x, W_qkv, b_qkv, W_out, b_out, core_id):
    g = core_id // CPG
    lane = core_id % CPG
    h0 = lane * HPC
    r = slice(h0 * DH, (h0 + HPC) * DH)
    Wq = W_qkv[0 * DM : 1 * DM, :][r, :]
    Wk = W_qkv[1 * DM : 2 * DM, :][r, :]
    Wv = W_qkv[2 * DM : 3 * DM, :][r, :]
    bq = b_qkv[0 * DM + h0 * DH : 0 * DM + (h0 + HPC) * DH]
    bk = b_qkv[1 * DM + h0 * DH : 1 * DM + (h0 + HPC) * DH]
    bv_ = b_qkv[2 * DM + h0 * DH : 2 * DM + (h0 + HPC) * DH]
    return {
        "xT": np.ascontiguousarray(x[g].T.astype(ml_dtypes.bfloat16)),
        "wqkT": np.ascontiguousarray(
            np.concatenate([Wq.T, Wk.T], axis=1).astype(ml_dtypes.bfloat16)
        ),
        "wvT": np.ascontiguousarray(Wv.T.astype(ml_dtypes.bfloat16)),
        "woT": np.ascontiguousarray(W_out[:, r].T.astype(ml_dtypes.bfloat16)),
        "bqk": np.concatenate([bq, bk]).reshape(2 * DQ, 1).astype(np.float32),
        "bv": np.ascontiguousarray(
            np.broadcast_to(bv_.reshape(1, DQ), (128, DQ)).astype(np.float32)
        ),
    }


def kernel(x, W_qkv, b_qkv, W_out, b_out, _trace=False):
    x = np.asarray(x)
    W_qkv = np.asarray(W_qkv)
    b_qkv = np.asarray(b_qkv)
    W_out = np.asarray(W_out)
    b_out = np.asarray(b_out)

    nc = _get_compiled()
    in_maps = [
        _prep_core_inputs(x, W_qkv, b_qkv, W_out, b_out, c) for c in range(N_CORES)
    ]
    res = run_bass_kernel_spmd(nc, in_maps, list(range(N_CORES)), trace=_trace)

    out = np.empty((B, S, DM), dtype=np.float32)
    for g in range(B):
        acc = res.results[g * CPG]["out"].astype(np.float32)
        for lane in range(1, CPG):
            acc = acc + res.results[g * CPG + lane]["out"]
        out[g] = acc + b_out[None, :].astype(np.float32)

    if _trace:
        kernel.last_exec_time_ns = res.exec_time_ns
        kernel.last_results = res
    return out


# revision 32
# speedup vs baseline: 1.8193x; 1.0128x over previous
"""Multi-head causal self-attention (torch nn.MultiheadAttention semantics)
on 8 Trainium2 NeuronCores.

Problem: x [2, 2048, 1024], 16 heads, head dim 64, fp32, causal, p_drop=0.

Sharding: 2 batch groups x 4-way head tensor-parallel.
  core c: batch b = c // 4, heads [lane*4, lane*4+4) with lane = c % 4.
Each core computes q/k/v projections for its 4 heads, flash-style causal
attention (S^T score layout, no-max softmax), and its partial out-projection.
The host sums the 4 partials per batch and adds b_out.

Performance structure (v2): the whole per-core program is emitted as ONE
software-pipelined instruction stream so the PE sequencer never blocks on a
semaphore. A blocked PE sequencer serializes the next LDWEIGHTS behind the
current matmul AND holds the PE at the 1.2 GHz mid p-state; back-to-back
matmuls hide LDWEIGHTS entirely and ramp the PE to 2.4 GHz after ~3 us
(measured: 230 ns vs 800+ ns per 512-row f32r matmul).

  - Attention runs in 2-head passes (scores psum "sc" 4x[128,512] rotation,
    PV accumulators "pa" 2x[65,512], projections/out-proj "po" 2x[128,512]
    = exactly 8 PSUM banks).
  - q/k/v projections of span sp+1 and out-projection of span sp-1 are
    generators whose matmuls are pulled 2-per-attention-step as PE filler
    between a step's score matmuls and the previous step's PV matmuls,
    covering the exp (Act) latency.
  - exp outputs P^T in bf16 (Act speed is dtype-independent; DVE tri-mask
    gets 2x; PV runs 1 cycle/row at any width, enabling causal trimming of
    the diagonal blocks on both the scores and PV matmuls).
  - softmax denominator rides as a 65th "ones" column of V'; normalization
    is reciprocal_approx_fast + gpsimd partition-broadcast + DVE mul.
"""

import ml_dtypes
import numpy as np
from contextlib import ExitStack

import concourse.bass as bass
import concourse.tile as tile
from concourse import bacc, mybir
from concourse.bass_utils import run_bass_kernel_spmd

F32 = mybir.dt.float32
F32R = mybir.dt.float32r
BF16 = mybir.dt.bfloat16
AF = mybir.ActivationFunctionType

B = 2
S = 2048
DM = 1024
N_HEADS = 16
DH = 64
N_CORES = 8
CPG = 4                    # cores per group (tensor-parallel width)
HPC = N_HEADS // CPG       # 4 heads per core
DQ = HPC * DH              # 256
SPAN = 512
SB = 128
NDM = DM // 128            # 8 x/weight chunks
NSPAN = S // SPAN          # 4
SBS = SPAN // SB           # 4
NSB = S // SB              # 16
NQK = 2 * DQ // 128        # 4 qkT row tiles
NHD = DQ // 128            # 2 OT row tiles
VW = DH + 1                # 65: per-head V plus ones column
OW = 512
NOUT = DM // OW            # 2


def _declare_io(nc):
    t = {}
    t["xT"] = nc.dram_tensor("xT", [DM, S], BF16, kind="ExternalInput").ap()
    t["wqkT"] = nc.dram_tensor("wqkT", [DM, 2 * DQ], BF16, kind="ExternalInput").ap()
    t["wvT"] = nc.dram_tensor("wvT", [DM, DQ], BF16, kind="ExternalInput").ap()
    t["woT"] = nc.dram_tensor("woT", [DQ, DM], BF16, kind="ExternalInput").ap()
    t["bqk"] = nc.dram_tensor("bqk", [2 * DQ, 1], F32, kind="ExternalInput").ap()
    t["bv"] = nc.dram_tensor("bv", [128, DQ], F32, kind="ExternalInput").ap()
    t["out"] = nc.dram_tensor("out", [S, DM], BF16, kind="ExternalOutput").ap()
    return t


def _build(ctx: ExitStack, tc: tile.TileContext, io: dict):
    nc = tc.nc

    const = ctx.enter_context(tc.tile_pool(name="const", bufs=1))
    work = ctx.enter_context(tc.tile_pool(name="work", bufs=1))
    psum = ctx.enter_context(tc.tile_pool(name="psum", bufs=1, space="PSUM"))

    # ---- PE warm-up: dependency-free matmuls on a memset tile keep the PE
    # busy through the initial input-DMA wait, so the p-state ramp (0.65 ->
    # 1.2 -> 2.4 GHz after 3us continuous) completes before real work ----
    dummy = const.tile([128, SPAN], BF16, name="dummy")
    nc.gpsimd.memset(dummy[:], 0.0)
    pwarm = psum.tile([128, SPAN], F32, name="pwarm", tag="sc", bufs=2)
    for _ in range(24):
        nc.tensor.matmul(pwarm[:], dummy[:, 0:128], dummy[:], start=True, stop=True,
                         skip_group_check=True)

    # ---- input DMAs: ordered so phase1(span 0) can start ~immediately ----
    wv = const.tile([128, NDM * DQ], BF16, name="wv")
    nc.scalar.dma_start(
        wv[:].rearrange("p (c w) -> p c w", w=DQ),
        io["wvT"].rearrange("(c p) w -> p c w", p=128),
    )
    bv = const.tile([128, DQ], F32, name="bv")
    nc.scalar.dma_start(bv[:], io["bv"][:])
    bqk = const.tile([128, NQK], F32, name="bqk")
    for obi in range(NQK):
        nc.scalar.dma_start(bqk[:, obi : obi + 1], io["bqk"][obi * 128 : (obi + 1) * 128, :])

    wqk = const.tile([128, NDM * 2 * DQ], BF16, name="wqk")
    xT = const.tile([128, NDM * S], BF16, name="xT")
    # half-batched loads: chunks 0-3 then 4-7, x-span0 ahead of wqk
    for lo in (0, 4):
        nc.sync.dma_start(
            xT[:].rearrange("p (c s) -> p c s", s=S)[:, lo : lo + 4, 0:SPAN],
            io["xT"].rearrange("(c p) s -> p c s", p=128)[:, lo : lo + 4, 0:SPAN],
        )
        nc.sync.dma_start(
            wqk[:].rearrange("p (c w) -> p c w", w=2 * DQ)[:, lo : lo + 4, :],
            io["wqkT"].rearrange("(c p) w -> p c w", p=128)[:, lo : lo + 4, :],
        )
    for sp in range(1, NSPAN):
        nc.sync.dma_start(
            xT[:].rearrange("p (c s) -> p c s", s=S)[:, :, sp * SPAN : (sp + 1) * SPAN],
            io["xT"].rearrange("(c p) s -> p c s", p=128)[:, :, sp * SPAN : (sp + 1) * SPAN],
        )
    wo = const.tile([128, NHD * DM], BF16, name="wo")
    nc.scalar.dma_start(
        wo[:].rearrange("p (c w) -> p c w", w=DM),
        io["woT"].rearrange("(c p) w -> p c w", p=128),
    )

    # causal triangle for one diagonal 128x128 sub-block, duplicated for the
    # 2-head strided multiply: tri2[r, j*128+c] = (c - r >= 0)
    tri2 = const.tile([128, 2 * SB], BF16, name="tri2")
    nc.gpsimd.memset(tri2[:], 1.0)
    for half in range(2):
        nc.gpsimd.affine_select(
            out=tri2[:, half * SB : (half + 1) * SB],
            in_=tri2[:, half * SB : (half + 1) * SB],
            compare_op=mybir.AluOpType.is_ge,
            fill=0.0,
            base=0,
            pattern=[[1, SB]],
            channel_multiplier=-1,
        )

    # ---- persistent activations ----
    qkT = [const.tile([128, S], BF16, name=f"qkT{b}") for b in range(NQK)]
    vp = [const.tile([128, HPC * VW], BF16, name=f"vp{sb}") for sb in range(NSB)]
    OT = [const.tile([128, S], BF16, name=f"OT{c}") for c in range(NHD)]

    vp_ready = [False] * NSB
    p1b = {}  # span -> generator for that span's pass-B qk projections

    # ---- filler generators (each yield = one PE matmul emitted) ----
    def gen_phase1(sp, obs):
        for obi in obs:
            pqk = psum.tile([128, SPAN], F32, name=f"pqk_{sp}_{obi}", tag="po", bufs=2)
            for c in range(NDM):
                nc.tensor.matmul(
                    pqk[:],
                    wqk[:, c * 2 * DQ + obi * 128 : c * 2 * DQ + (obi + 1) * 128],
                    xT[:, c * S + sp * SPAN : c * S + (sp + 1) * SPAN],
                    start=(c == 0),
                    stop=(c == NDM - 1),
                    skip_group_check=True,
                )
                if c < NDM - 1:
                    yield
            nc.vector.tensor_scalar_add(
                qkT[obi][:, sp * SPAN : (sp + 1) * SPAN], pqk[:], bqk[:, obi : obi + 1]
            )
            yield

    def gen_phase2(sp):
        for sb in range(sp * SBS, (sp + 1) * SBS):
            pv = psum.tile([128, DQ], F32, name=f"pv_{sb}", tag="po", bufs=2)
            for c in range(NDM):
                nc.tensor.matmul(
                    pv[:],
                    xT[:, c * S + sb * SB : c * S + (sb + 1) * SB],
                    wv[:, c * DQ : (c + 1) * DQ],
                    start=(c == 0),
                    stop=(c == NDM - 1),
                    skip_group_check=True,
                )
                if c < NDM - 1:
                    yield
            vdst = vp[sb][:, :].rearrange("p (h w) -> p h w", w=VW)[:, :, 0:DH]
            nc.vector.tensor_add(
                vdst,
                pv[:].rearrange("p (h d) -> p h d", d=DH),
                bv[:].rearrange("p (h d) -> p h d", d=DH),
            )
            ones = vp[sb][:, DH : HPC * VW : VW]
            nc.vector.memset(ones, 1.0)
            vp_ready[sb] = True
            yield

    def gen_outproj(sp):
        for qb in range(sp * SBS, (sp + 1) * SBS):
            ob_t = work.tile([128, DM], BF16, name=f"ob_{qb}", tag="ob", bufs=2)
            for nh in range(NOUT):
                pot = psum.tile([128, OW], F32, name=f"pot_{qb}_{nh}", tag="po", bufs=2)
                for c in range(NHD):
                    nc.tensor.matmul(
                        pot[:],
                        OT[c][:, qb * SB : (qb + 1) * SB],
                        wo[:, c * DM + nh * OW : c * DM + (nh + 1) * OW],
                        start=(c == 0),
                        stop=(c == NHD - 1),
                        skip_group_check=True,
                    )
                    yield
                if nh % 2 == 0:
                    nc.vector.tensor_copy(ob_t[:, nh * OW : (nh + 1) * OW], pot[:])
                else:
                    nc.scalar.copy(ob_t[:, nh * OW : (nh + 1) * OW], pot[:])
            nc.sync.dma_start(io["out"][qb * SB : (qb + 1) * SB, :], ob_t[:])
            yield

    # Two-level filler queue: "prio" holds next-span projections (must finish
    # before that span's attention), "bulk" holds out-projections (no deadline
    # until the end — saved to keep the PE fed through span 3's long passes).
    prio = []
    bulk = []

    def pull(n):
        k = 0
        while k < n:
            q = prio if prio else bulk
            if not q:
                return
            try:
                next(q[0])
                k += 1
            except StopIteration:
                q.pop(0)

    def exhaust(g):
        while True:
            try:
                next(g)
            except StopIteration:
                break
        if g in prio:
            prio.remove(g)
        if g in bulk:
            bulk.remove(g)

    def drain_all():
        while prio or bulk:
            pull(64)

    # ---- attention for one span, two 2-head passes ----
    def attention_span(sp):
        nsb = (sp + 1) * SBS
        for p in range(2):
            hA = 2 * p
            pts = {}
            pos = {}

            def emit_scores(i):
                sb = i
                d = sb - sp * SBS
                offc = offe = 0 if d < 1 else 128 * d
                pt = work.tile(
                    [128, 2 * SPAN], BF16, name=f"pt_{sp}_{p}_{i}", tag="pt", bufs=3
                )
                pts[i] = (pt, offe)
                s_t = psum.tile(
                    [128, 2 * SPAN], F32, name=f"ps_{sp}_{p}_{i}", tag="sc", bufs=2
                )
                for j in range(2):
                    h = hA + j
                    qt = qkT[h // 2]
                    kt = qkT[NQK // 2 + h // 2]
                    qrow = (h % 2) * 64
                    nc.tensor.matmul(
                        s_t[:, j * SPAN + offe : (j + 1) * SPAN],
                        kt[qrow : qrow + 64, sb * SB : (sb + 1) * SB],
                        qt[qrow : qrow + 64, sp * SPAN + offe : (sp + 1) * SPAN],
                        start=True,
                        stop=True,
                        skip_group_check=True,
                    )
                # one exp covers both heads' (trimmed) score blocks
                nc.scalar.activation(
                    pt[:].rearrange("p (j w) -> p j w", w=SPAN)[:, :, offe:SPAN],
                    s_t[:].rearrange("p (j w) -> p j w", w=SPAN)[:, :, offe:SPAN],
                    AF.Exp,
                    scale=0.125,
                )
                if d >= 0:
                    ptv = pt[:].rearrange("p (j w) -> p j w", w=SPAN)[
                        :, :, 128 * d : 128 * (d + 1)
                    ]
                    nc.vector.tensor_mul(
                        ptv, ptv, tri2[:].rearrange("p (j w) -> p j w", w=SB)
                    )

            def emit_pv(i):
                sb = i
                # emission-order guard: vp[sb]'s writer must be emitted first
                while not vp_ready[sb]:
                    pull(8)
                d = sb - sp * SBS
                offe = 0 if d < 1 else 128 * d
                pt, _ = pts.pop(i)
                for j in range(2):
                    h = hA + j
                    if i == 0:
                        pos[h] = psum.tile(
                            [VW, SPAN], F32, name=f"po_{sp}_{h}", tag="pa", bufs=2
                        )
                    dst = pos[h][:, offe:SPAN] if offe else pos[h][:]
                    nc.tensor.matmul(
                        dst,
                        vp[sb][:, h * VW : (h + 1) * VW],
                        pt[:, j * SPAN + offe : (j + 1) * SPAN],
                        start=(i == 0),
                        stop=(i == nsb - 1),
                        skip_group_check=True,
                    )

            if p == 1 and sp in p1b:
                exhaust(p1b[sp])  # pass B scores need qkT obs 1,3 of this span
            pull(4)
            # PV lags scores by 2 steps: both PE gates (scores' WAR on exp,
            # PV's RAW on exp) get a >=2-cycle window, so semaphore jitter
            # never stalls the PE sequencer (which would reset the p-state).
            for i in range(nsb + 2):
                if i < nsb:
                    emit_scores(i)
                pull(8 if i % 4 == 0 else 0)
                if i >= 2:
                    emit_pv(i - 2)

            # normalization: out^T rows 0..63 divided by the denominator row
            for j in range(2):
                h = hA + j
                den = work.tile([1, SPAN], F32, name=f"den_{sp}_{h}", tag="den", bufs=2)
                nc.vector.tensor_copy(den[0:1, :], pos[h][VW - 1 : VW, :])
                rden = work.tile([1, SPAN], F32, name=f"rden_{sp}_{h}", tag="rden", bufs=2)
                nc.vector.reciprocal_approx_fast(rden[0:1, :], den[0:1, :])
                recb = work.tile([DH, SPAN], F32, name=f"recb_{sp}_{h}", tag="recb", bufs=2)
                nc.gpsimd.partition_broadcast(recb[:], rden[0:1, :])
                nc.vector.tensor_mul(
                    OT[h // 2][(h % 2) * DH : (h % 2 + 1) * DH, sp * SPAN : (sp + 1) * SPAN],
                    pos[h][0:DH, :],
                    recb[:],
                )

    # ---- span-0 projections, contraction-chunk OUTER so each x/w chunk is
    # consumed as its DMA lands (the attention "sc" and "po" psum banks are
    # free this early, giving 4 qk + 2x2 v accumulators) ----
    # pqk0: ob pairs packed into 2-bank "sc" tiles (independent accumulation
    # groups may share a TILE but never a BANK — halves sit in separate banks)
    pqk0 = [psum.tile([128, 2 * SPAN], F32, name=f"pqk0_{i}", tag="sc", bufs=2) for i in range(2)]
    pv0 = [
        psum.tile([128, DQ], F32, name=f"pv0_{sb}", tag=("po" if sb < 2 else "pa"), bufs=2)
        for sb in range(SBS)
    ]
    for c in range(NDM):
        for obi in range(NQK):
            nc.tensor.matmul(
                pqk0[obi // 2][:, (obi % 2) * SPAN : (obi % 2 + 1) * SPAN],
                wqk[:, c * 2 * DQ + obi * 128 : c * 2 * DQ + (obi + 1) * 128],
                xT[:, c * S : c * S + SPAN],
                start=(c == 0),
                stop=(c == NDM - 1),
                skip_group_check=True,
            )
        for sb in range(SBS):
            nc.tensor.matmul(
                pv0[sb][:],
                xT[:, c * S + sb * SB : c * S + (sb + 1) * SB],
                wv[:, c * DQ : (c + 1) * DQ],
                start=(c == 0),
                stop=(c == NDM - 1),
                skip_group_check=True,
            )
    for obi in (0, 2, 1, 3):  # pass-A tiles (q heads 0/1, k heads 0/1) first
        nc.vector.tensor_scalar_add(
            qkT[obi][:, 0:SPAN],
            pqk0[obi // 2][:, (obi % 2) * SPAN : (obi % 2 + 1) * SPAN],
            bqk[:, obi : obi + 1],
        )
    for sb in range(SBS):
        vdst = vp[sb][:, :].rearrange("p (h w) -> p h w", w=VW)[:, :, 0:DH]
        nc.vector.tensor_add(
            vdst,
            pv0[sb][:].rearrange("p (h d) -> p h d", d=DH),
            bv[:].rearrange("p (h d) -> p h d", d=DH),
        )
        nc.vector.memset(vp[sb][:, DH : HPC * VW : VW], 1.0)
        vp_ready[sb] = True
    for sp in range(NSPAN):
        if sp + 1 < NSPAN:
            p1a = gen_phase1(sp + 1, (0, 2))
            p1b[sp + 1] = gen_phase1(sp + 1, (1, 3))
            prio.append(p1a)
            prio.append(p1b[sp + 1])
            prio.append(gen_phase2(sp + 1))
        attention_span(sp)
        if sp + 1 < NSPAN:
            exhaust(p1a)  # next span's pass A needs qkT obs 0,2 emitted
        if sp < 2:
            bulk.append(gen_outproj(sp))
    # op(2) held back: its 16 ready-to-run matmuls keep the PE hot across the
    # final normalization chain; op(3) follows right behind.
    bulk.append(gen_outproj(2))
    bulk.append(gen_outproj(3))
    drain_all()


_NC_CACHE = {}


def _get_compiled():
    if "nc" not in _NC_CACHE:
        nc = bacc.Bacc(
            "TRN2", target_bir_lowering=False, debug=False, num_devices=N_CORES
        )
        io = _declare_io(nc)
        with tile.TileContext(nc) as tc, ExitStack() as ctx:
            _build(ctx, tc, io)
        nc.compile()
        _NC_CACHE["nc"] = nc
    return _NC_CACHE["nc"]


def _prep_core_inputs(x, W_qkv, b_qkv, W_out, b_out, core_id):
    g = core_id // CPG
    lane = core_id % CPG
    h0 = lane * HPC
    r = slice(h0 * DH, (h0 + HPC) * DH)
    Wq = W_qkv[0 * DM : 1 * DM, :][r, :]
    Wk = W_qkv[1 * DM : 2 * DM, :][r, :]
    Wv = W_qkv[2 * DM : 3 * DM, :][r, :]
    bq = b_qkv[0 * DM + h0 * DH : 0 * DM + (h0 + HPC) * DH]
    bk = b_qkv[1 * DM + h0 * DH : 1 * DM + (h0 + HPC) * DH]
    bv_ = b_qkv[2 * DM + h0 * DH : 2 * DM + (h0 + HPC) * DH]
    return {
        "xT": np.ascontiguousarray(x[g].T.astype(ml_dtypes.bfloat16)),
        "wqkT": np.ascontiguousarray(
            np.concatenate([Wq.T, Wk.T], axis=1).astype(ml_dtypes.bfloat16)
        ),
        "wvT": np.ascontiguousarray(Wv.T.astype(ml_dtypes.bfloat16)),
        "woT": np.ascontiguousarray(W_out[:, r].T.astype(ml_dtypes.bfloat16)),
        "bqk": np.concatenate([bq, bk]).reshape(2 * DQ, 1).astype(np.float32),
        "bv": np.ascontiguousarray(
            np.broadcast_to(bv_.reshape(1, DQ), (128, DQ)).astype(np.float32)
        ),
    }


def kernel(x, W_qkv, b_qkv, W_out, b_out, _trace=False):
    x = np.asarray(x)
    W_qkv = np.asarray(W_qkv)
    b_qkv = np.asarray(b_qkv)
    W_out = np.asarray(W_out)
    b_out = np.asarray(b_out)

    nc = _get_compiled()
    in_maps = [
        _prep_core_inputs(x, W_qkv, b_qkv, W_out, b_out, c) for c in range(N_CORES)
    ]
    res = run_bass_kernel_spmd(nc, in_maps, list(range(N_CORES)), trace=_trace)

    out = np.empty((B, S, DM), dtype=np.float32)
    for g in range(B):
        acc = res.results[g * CPG]["out"].astype(np.float32)
        for lane in range(1, CPG):
            acc = acc + res.results[g * CPG + lane]["out"]
        out[g] = acc + b_out[None, :].astype(np.float32)

    if _trace:
        kernel.last_exec_time_ns = res.exec_time_ns
        kernel.last_results = res
    return out


# revision 33
# speedup vs baseline: 1.8241x; 1.0026x over previous
"""Multi-head causal self-attention (torch nn.MultiheadAttention semantics)
on 8 Trainium2 NeuronCores.

Problem: x [2, 2048, 1024], 16 heads, head dim 64, fp32, causal, p_drop=0.

Sharding: 2 batch groups x 4-way head tensor-parallel.
  core c: batch b = c // 4, heads [lane*4, lane*4+4) with lane = c % 4.
Each core computes q/k/v projections for its 4 heads, flash-style causal
attention (S^T score layout, no-max softmax), and its partial out-projection.
The host sums the 4 partials per batch and adds b_out.

Performance structure (v2): the whole per-core program is emitted as ONE
software-pipelined instruction stream so the PE sequencer never blocks on a
semaphore. A blocked PE sequencer serializes the next LDWEIGHTS behind the
current matmul AND holds the PE at the 1.2 GHz mid p-state; back-to-back
matmuls hide LDWEIGHTS entirely and ramp the PE to 2.4 GHz after ~3 us
(measured: 230 ns vs 800+ ns per 512-row f32r matmul).

  - Attention runs in 2-head passes (scores psum "sc" 4x[128,512] rotation,
    PV accumulators "pa" 2x[65,512], projections/out-proj "po" 2x[128,512]
    = exactly 8 PSUM banks).
  - q/k/v projections of span sp+1 and out-projection of span sp-1 are
    generators whose matmuls are pulled 2-per-attention-step as PE filler
    between a step's score matmuls and the previous step's PV matmuls,
    covering the exp (Act) latency.
  - exp outputs P^T in bf16 (Act speed is dtype-independent; DVE tri-mask
    gets 2x; PV runs 1 cycle/row at any width, enabling causal trimming of
    the diagonal blocks on both the scores and PV matmuls).
  - softmax denominator rides as a 65th "ones" column of V'; normalization
    is reciprocal_approx_fast + gpsimd partition-broadcast + DVE mul.
"""

import ml_dtypes
import numpy as np
from contextlib import ExitStack

import concourse.bass as bass
import concourse.tile as tile
from concourse import bacc, mybir
from concourse.bass_utils import run_bass_kernel_spmd

F32 = mybir.dt.float32
F32R = mybir.dt.float32r
BF16 = mybir.dt.bfloat16
AF = mybir.ActivationFunctionType

B = 2
S = 2048
DM = 1024
N_HEADS = 16
DH = 64
N_CORES = 8
CPG = 4                    # cores per group (tensor-parallel width)
HPC = N_HEADS // CPG       # 4 heads per core
DQ = HPC * DH              # 256
SPAN = 512
SB = 128
NDM = DM // 128            # 8 x/weight chunks
NSPAN = S // SPAN          # 4
SBS = SPAN // SB           # 4
NSB = S // SB              # 16
NQK = 2 * DQ // 128        # 4 qkT row tiles
NHD = DQ // 128            # 2 OT row tiles
VW = DH + 1                # 65: per-head V plus ones column
OW = 512
NOUT = DM // OW            # 2


def _declare_io(nc):
    t = {}
    t["xT"] = nc.dram_tensor("xT", [DM, S], BF16, kind="ExternalInput").ap()
    t["wqkT"] = nc.dram_tensor("wqkT", [DM, 2 * DQ], BF16, kind="ExternalInput").ap()
    t["wvT"] = nc.dram_tensor("wvT", [DM, DQ], BF16, kind="ExternalInput").ap()
    t["woT"] = nc.dram_tensor("woT", [DQ, DM], BF16, kind="ExternalInput").ap()
    t["bqk"] = nc.dram_tensor("bqk", [2 * DQ, 1], F32, kind="ExternalInput").ap()
    t["bv"] = nc.dram_tensor("bv", [128, DQ], F32, kind="ExternalInput").ap()
    t["out"] = nc.dram_tensor("out", [S, DM], BF16, kind="ExternalOutput").ap()
    return t


def _build(ctx: ExitStack, tc: tile.TileContext, io: dict):
    nc = tc.nc

    const = ctx.enter_context(tc.tile_pool(name="const", bufs=1))
    work = ctx.enter_context(tc.tile_pool(name="work", bufs=1))
    psum = ctx.enter_context(tc.tile_pool(name="psum", bufs=1, space="PSUM"))

    # ---- PE warm-up: dependency-free matmuls on a memset tile keep the PE
    # busy through the initial input-DMA wait, so the p-state ramp (0.65 ->
    # 1.2 -> 2.4 GHz after 3us continuous) completes before real work ----
    dummy = const.tile([128, SPAN], BF16, name="dummy")
    nc.gpsimd.memset(dummy[:], 0.0)
    pwarm = psum.tile([128, SPAN], F32, name="pwarm", tag="sc", bufs=2)
    for _ in range(12):
        nc.tensor.matmul(pwarm[:], dummy[:, 0:128], dummy[:], start=True, stop=True,
                         skip_group_check=True)

    # ---- input DMAs: ordered so phase1(span 0) can start ~immediately ----
    wv = const.tile([128, NDM * DQ], BF16, name="wv")
    nc.scalar.dma_start(
        wv[:].rearrange("p (c w) -> p c w", w=DQ),
        io["wvT"].rearrange("(c p) w -> p c w", p=128),
    )
    bv = const.tile([128, DQ], F32, name="bv")
    nc.scalar.dma_start(bv[:], io["bv"][:])
    bqk = const.tile([128, NQK], F32, name="bqk")
    for obi in range(NQK):
        nc.scalar.dma_start(bqk[:, obi : obi + 1], io["bqk"][obi * 128 : (obi + 1) * 128, :])

    wqk = const.tile([128, NDM * 2 * DQ], BF16, name="wqk")
    xT = const.tile([128, NDM * S], BF16, name="xT")
    # half-batched loads: chunks 0-3 then 4-7, x-span0 ahead of wqk
    for lo in (0, 4):
        nc.sync.dma_start(
            xT[:].rearrange("p (c s) -> p c s", s=S)[:, lo : lo + 4, 0:SPAN],
            io["xT"].rearrange("(c p) s -> p c s", p=128)[:, lo : lo + 4, 0:SPAN],
        )
        nc.sync.dma_start(
            wqk[:].rearrange("p (c w) -> p c w", w=2 * DQ)[:, lo : lo + 4, :],
            io["wqkT"].rearrange("(c p) w -> p c w", p=128)[:, lo : lo + 4, :],
        )
    for sp in range(1, NSPAN):
        nc.sync.dma_start(
            xT[:].rearrange("p (c s) -> p c s", s=S)[:, :, sp * SPAN : (sp + 1) * SPAN],
            io["xT"].rearrange("(c p) s -> p c s", p=128)[:, :, sp * SPAN : (sp + 1) * SPAN],
        )
    wo = const.tile([128, NHD * DM], BF16, name="wo")
    nc.scalar.dma_start(
        wo[:].rearrange("p (c w) -> p c w", w=DM),
        io["woT"].rearrange("(c p) w -> p c w", p=128),
    )

    # causal triangle for one diagonal 128x128 sub-block, duplicated for the
    # 2-head strided multiply: tri2[r, j*128+c] = (c - r >= 0)
    tri2 = const.tile([128, 2 * SB], BF16, name="tri2")
    nc.gpsimd.memset(tri2[:], 1.0)
    for half in range(2):
        nc.gpsimd.affine_select(
            out=tri2[:, half * SB : (half + 1) * SB],
            in_=tri2[:, half * SB : (half + 1) * SB],
            compare_op=mybir.AluOpType.is_ge,
            fill=0.0,
            base=0,
            pattern=[[1, SB]],
            channel_multiplier=-1,
        )

    # ---- persistent activations ----
    qkT = [const.tile([128, S], BF16, name=f"qkT{b}") for b in range(NQK)]
    vp = [const.tile([128, HPC * VW], BF16, name=f"vp{sb}") for sb in range(NSB)]
    OT = [const.tile([128, S], BF16, name=f"OT{c}") for c in range(NHD)]

    vp_ready = [False] * NSB
    p1b = {}  # span -> generator for that span's pass-B qk projections

    # ---- filler generators (each yield = one PE matmul emitted) ----
    def gen_phase1(sp, obs):
        for obi in obs:
            pqk = psum.tile([128, SPAN], F32, name=f"pqk_{sp}_{obi}", tag="po", bufs=2)
            for c in range(NDM):
                nc.tensor.matmul(
                    pqk[:],
                    wqk[:, c * 2 * DQ + obi * 128 : c * 2 * DQ + (obi + 1) * 128],
                    xT[:, c * S + sp * SPAN : c * S + (sp + 1) * SPAN],
                    start=(c == 0),
                    stop=(c == NDM - 1),
                    skip_group_check=True,
                )
                if c < NDM - 1:
                    yield
            nc.vector.tensor_scalar_add(
                qkT[obi][:, sp * SPAN : (sp + 1) * SPAN], pqk[:], bqk[:, obi : obi + 1]
            )
            yield

    def gen_phase2(sp):
        for sb in range(sp * SBS, (sp + 1) * SBS):
            pv = psum.tile([128, DQ], F32, name=f"pv_{sb}", tag="po", bufs=2)
            for c in range(NDM):
                nc.tensor.matmul(
                    pv[:],
                    xT[:, c * S + sb * SB : c * S + (sb + 1) * SB],
                    wv[:, c * DQ : (c + 1) * DQ],
                    start=(c == 0),
                    stop=(c == NDM - 1),
                    skip_group_check=True,
                )
                if c < NDM - 1:
                    yield
            vdst = vp[sb][:, :].rearrange("p (h w) -> p h w", w=VW)[:, :, 0:DH]
            nc.vector.tensor_add(
                vdst,
                pv[:].rearrange("p (h d) -> p h d", d=DH),
                bv[:].rearrange("p (h d) -> p h d", d=DH),
            )
            ones = vp[sb][:, DH : HPC * VW : VW]
            nc.vector.memset(ones, 1.0)
            vp_ready[sb] = True
            yield

    def gen_outproj(sp, act_copies=False):
        for qb in range(sp * SBS, (sp + 1) * SBS):
            ob_t = work.tile([128, DM], BF16, name=f"ob_{qb}", tag="ob", bufs=2)
            for nh in range(NOUT):
                pot = psum.tile([128, OW], F32, name=f"pot_{qb}_{nh}", tag="po", bufs=2)
                for c in range(NHD):
                    nc.tensor.matmul(
                        pot[:],
                        OT[c][:, qb * SB : (qb + 1) * SB],
                        wo[:, c * DM + nh * OW : c * DM + (nh + 1) * OW],
                        start=(c == 0),
                        stop=(c == NHD - 1),
                        skip_group_check=True,
                    )
                    yield
                if nh % 2 == 1 and act_copies:
                    nc.scalar.copy(ob_t[:, nh * OW : (nh + 1) * OW], pot[:])
                else:
                    nc.vector.tensor_copy(ob_t[:, nh * OW : (nh + 1) * OW], pot[:])
            nc.sync.dma_start(io["out"][qb * SB : (qb + 1) * SB, :], ob_t[:])
            yield

    # Two-level filler queue: "prio" holds next-span projections (must finish
    # before that span's attention), "bulk" holds out-projections (no deadline
    # until the end — saved to keep the PE fed through span 3's long passes).
    prio = []
    bulk = []

    def pull(n):
        k = 0
        while k < n:
            q = prio if prio else bulk
            if not q:
                return
            try:
                next(q[0])
                k += 1
            except StopIteration:
                q.pop(0)

    def exhaust(g):
        while True:
            try:
                next(g)
            except StopIteration:
                break
        if g in prio:
            prio.remove(g)
        if g in bulk:
            bulk.remove(g)

    def drain_all():
        while prio or bulk:
            pull(64)

    # ---- attention for one span, two 2-head passes ----
    def attention_span(sp):
        nsb = (sp + 1) * SBS
        for p in range(2):
            hA = 2 * p
            pts = {}
            pos = {}

            def emit_scores(i):
                sb = i
                d = sb - sp * SBS
                offc = offe = 0 if d < 1 else 128 * d
                pt = work.tile(
                    [128, 2 * SPAN], BF16, name=f"pt_{sp}_{p}_{i}", tag="pt", bufs=3
                )
                pts[i] = (pt, offe)
                s_t = psum.tile(
                    [128, 2 * SPAN], F32, name=f"ps_{sp}_{p}_{i}", tag="sc", bufs=2
                )
                for j in range(2):
                    h = hA + j
                    qt = qkT[h // 2]
                    kt = qkT[NQK // 2 + h // 2]
                    qrow = (h % 2) * 64
                    nc.tensor.matmul(
                        s_t[:, j * SPAN + offe : (j + 1) * SPAN],
                        kt[qrow : qrow + 64, sb * SB : (sb + 1) * SB],
                        qt[qrow : qrow + 64, sp * SPAN + offe : (sp + 1) * SPAN],
                        start=True,
                        stop=True,
                        skip_group_check=True,
                    )
                # one exp covers both heads' (trimmed) score blocks
                nc.scalar.activation(
                    pt[:].rearrange("p (j w) -> p j w", w=SPAN)[:, :, offe:SPAN],
                    s_t[:].rearrange("p (j w) -> p j w", w=SPAN)[:, :, offe:SPAN],
                    AF.Exp,
                    scale=0.125,
                )
                if d >= 0:
                    ptv = pt[:].rearrange("p (j w) -> p j w", w=SPAN)[
                        :, :, 128 * d : 128 * (d + 1)
                    ]
                    nc.vector.tensor_mul(
                        ptv, ptv, tri2[:].rearrange("p (j w) -> p j w", w=SB)
                    )

            def emit_pv(i):
                sb = i
                # emission-order guard: vp[sb]'s writer must be emitted first
                while not vp_ready[sb]:
                    pull(8)
                d = sb - sp * SBS
                offe = 0 if d < 1 else 128 * d
                pt, _ = pts.pop(i)
                for j in range(2):
                    h = hA + j
                    if i == 0:
                        pos[h] = psum.tile(
                            [VW, SPAN], F32, name=f"po_{sp}_{h}", tag="pa", bufs=2
                        )
                    dst = pos[h][:, offe:SPAN] if offe else pos[h][:]
                    nc.tensor.matmul(
                        dst,
                        vp[sb][:, h * VW : (h + 1) * VW],
                        pt[:, j * SPAN + offe : (j + 1) * SPAN],
                        start=(i == 0),
                        stop=(i == nsb - 1),
                        skip_group_check=True,
                    )

            if p == 1 and sp in p1b:
                exhaust(p1b[sp])  # pass B scores need qkT obs 1,3 of this span
            pull(4)
            # PV lags scores by 2 steps: both PE gates (scores' WAR on exp,
            # PV's RAW on exp) get a >=2-cycle window, so semaphore jitter
            # never stalls the PE sequencer (which would reset the p-state).
            for i in range(nsb + 2):
                if i < nsb:
                    emit_scores(i)
                pull(8 if i % 4 == 0 else 0)
                if i >= 2:
                    emit_pv(i - 2)

            # normalization: out^T rows 0..63 divided by the denominator row
            for j in range(2):
                h = hA + j
                den = work.tile([1, SPAN], F32, name=f"den_{sp}_{h}", tag="den", bufs=2)
                nc.vector.tensor_copy(den[0:1, :], pos[h][VW - 1 : VW, :])
                rden = work.tile([1, SPAN], F32, name=f"rden_{sp}_{h}", tag="rden", bufs=2)
                nc.vector.reciprocal_approx_fast(rden[0:1, :], den[0:1, :])
                recb = work.tile([DH, SPAN], F32, name=f"recb_{sp}_{h}", tag="recb", bufs=2)
                nc.gpsimd.partition_broadcast(recb[:], rden[0:1, :])
                nc.vector.tensor_mul(
                    OT[h // 2][(h % 2) * DH : (h % 2 + 1) * DH, sp * SPAN : (sp + 1) * SPAN],
                    pos[h][0:DH, :],
                    recb[:],
                )

    # ---- span-0 projections, contraction-chunk OUTER so each x/w chunk is
    # consumed as its DMA lands (the attention "sc" and "po" psum banks are
    # free this early, giving 4 qk + 2x2 v accumulators) ----
    # pqk0: ob pairs packed into 2-bank "sc" tiles (independent accumulation
    # groups may share a TILE but never a BANK — halves sit in separate banks)
    pqk0 = [psum.tile([128, 2 * SPAN], F32, name=f"pqk0_{i}", tag="sc", bufs=2) for i in range(2)]
    pv0 = [
        psum.tile([128, DQ], F32, name=f"pv0_{sb}", tag=("po" if sb < 2 else "pa"), bufs=2)
        for sb in range(SBS)
    ]
    for c in range(NDM):
        for obi in range(NQK):
            nc.tensor.matmul(
                pqk0[obi // 2][:, (obi % 2) * SPAN : (obi % 2 + 1) * SPAN],
                wqk[:, c * 2 * DQ + obi * 128 : c * 2 * DQ + (obi + 1) * 128],
                xT[:, c * S : c * S + SPAN],
                start=(c == 0),
                stop=(c == NDM - 1),
                skip_group_check=True,
            )
        for sb in range(SBS):
            nc.tensor.matmul(
                pv0[sb][:],
                xT[:, c * S + sb * SB : c * S + (sb + 1) * SB],
                wv[:, c * DQ : (c + 1) * DQ],
                start=(c == 0),
                stop=(c == NDM - 1),
                skip_group_check=True,
            )
    for obi in (0, 2, 1, 3):  # pass-A tiles (q heads 0/1, k heads 0/1) first
        nc.vector.tensor_scalar_add(
            qkT[obi][:, 0:SPAN],
            pqk0[obi // 2][:, (obi % 2) * SPAN : (obi % 2 + 1) * SPAN],
            bqk[:, obi : obi + 1],
        )
    for sb in range(SBS):
        vdst = vp[sb][:, :].rearrange("p (h w) -> p h w", w=VW)[:, :, 0:DH]
        nc.vector.tensor_add(
            vdst,
            pv0[sb][:].rearrange("p (h d) -> p h d", d=DH),
            bv[:].rearrange("p (h d) -> p h d", d=DH),
        )
        nc.vector.memset(vp[sb][:, DH : HPC * VW : VW], 1.0)
        vp_ready[sb] = True
    for sp in range(NSPAN):
        if sp + 1 < NSPAN:
            p1a = gen_phase1(sp + 1, (0, 2))
            p1b[sp + 1] = gen_phase1(sp + 1, (1, 3))
            prio.append(p1a)
            prio.append(p1b[sp + 1])
            prio.append(gen_phase2(sp + 1))
        attention_span(sp)
        if sp + 1 < NSPAN:
            exhaust(p1a)  # next span's pass A needs qkT obs 0,2 emitted
        if sp < 2:
            bulk.append(gen_outproj(sp))
    # op(2) held back: its 16 ready-to-run matmuls keep the PE hot across the
    # final normalization chain; op(3) follows right behind.
    bulk.append(gen_outproj(2, act_copies=True))
    bulk.append(gen_outproj(3, act_copies=True))
    drain_all()


_NC_CACHE = {}


def _get_compiled():
    if "nc" not in _NC_CACHE:
        nc = bacc.Bacc(
            "TRN2", target_bir_lowering=False, debug=False, num_devices=N_CORES
        )
        io = _declare_io(nc)
        with tile.TileContext(nc) as tc, ExitStack() as ctx:
            _build(ctx, tc, io)
        nc.compile()
        _NC_CACHE["nc"] = nc
    return _NC_CACHE["nc"]


def _prep_core_inputs(x, W_qkv, b_qkv, W_out, b_out, core_id):
    g = core_id // CPG
    lane = core_id % CPG
    h0 = lane * HPC
    r = slice(h0 * DH, (h0 + HPC) * DH)
    Wq = W_qkv[0 * DM : 1 * DM, :][r, :]
    Wk = W_qkv[1 * DM : 2 * DM, :][r, :]
    Wv = W_qkv[2 * DM : 3 * DM, :][r, :]
    bq = b_qkv[0 * DM + h0 * DH : 0 * DM + (h0 + HPC) * DH]
    bk = b_qkv[1 * DM + h0 * DH : 1 * DM + (h0 + HPC) * DH]
    bv_ = b_qkv[2 * DM + h0 * DH : 2 * DM + (h0 + HPC) * DH]
    return {
        "xT": np.ascontiguousarray(x[g].T.astype(ml_dtypes.bfloat16)),
        "wqkT": np.ascontiguousarray(
            np.concatenate([Wq.T, Wk.T], axis=1).astype(ml_dtypes.bfloat16)
        ),
        "wvT": np.ascontiguousarray(Wv.T.astype(ml_dtypes.bfloat16)),
        "woT": np.ascontiguousarray(W_out[:, r].T.astype(ml_dtypes.bfloat16)),
        "bqk": np.concatenate([bq, bk]).reshape(2 * DQ, 1).astype(np.float32),
        "bv": np.ascontiguousarray(
            np.broadcast_to(bv_.reshape(1, DQ), (128, DQ)).astype(np.float32)
        ),
    }


def kernel(x, W_qkv, b_qkv, W_out, b_out, _trace=False):
    x = np.asarray(x)
    W_qkv = np.asarray(W_qkv)
    b_qkv = np.asarray(b_qkv)
    W_out = np.asarray(W_out)
    b_out = np.asarray(b_out)

    nc = _get_compiled()
    in_maps = [
        _prep_core_inputs(x, W_qkv, b_qkv, W_out, b_out, c) for c in range(N_CORES)
    ]
    res = run_bass_kernel_spmd(nc, in_maps, list(range(N_CORES)), trace=_trace)

    out = np.empty((B, S, DM), dtype=np.float32)
    for g in range(B):
        acc = res.results[g * CPG]["out"].astype(np.float32)
        for lane in range(1, CPG):
            acc = acc + res.results[g * CPG + lane]["out"]
        out[g] = acc + b_out[None, :].astype(np.float32)

    if _trace:
        kernel.last_exec_time_ns = res.exec_time_ns
        kernel.last_results = res
    return out
